# revision 15
# baseline (speedup 1.0000x reference)
"""Trainium2 Bass kernel for nn_Block_39247411151159.

Sharding: 8 cores = 4 batches x 2 head-groups (4 heads each).
Core c handles batch b=c//2, head-group hg=c%2 (global heads 4*hg..4*hg+3).
One pairwise AllReduce mid-kernel sums the re-atten conv partials (etc_k);
the final squeeze-conv partials are summed on the host during unshard.

All matmuls run as float32r (TF32-like: full PE speed at N>=256, ~1e-4
relative error). Softmax over the etc axis runs with e on partitions and no
max-subtraction (scores here are O(2)); the denominator is pre-reduced over
the four e-tiles on the (otherwise idle) Pool engine, then replicated across
partitions by a single full-ones stationary matmul per 512-column chunk.

Schedule notes (all per-core):
 - vT runs nj-split so the first matmul only needs 2 DMA chunks; tiny
   zero-warmup matmuls start the PE p-state ramp at ~250ns.
 - etc_v runs per head with the re-conv (rk) accumulation steps interleaved
   right behind each head, so the AllReduce launches ~3us after the last
   etc_v matmul.  The AllReduce is chunked per d-tile over two DMA queues
   (SP hwdge + Pool swdge) to halve exposed issue latency.
 - etc_vT for all four heads is produced by PE transposes (pairs packed per
   PSUM bank) inside the AllReduce shadow, alongside the q conv, the
   avgpool-branch conv, and a K=1 ones-matmul that replicates mask*regular
   across partitions (replaces a pathologically slow broadcast DMA).
 - head 0's scores use a pre-scaled copy of QM (mix/sqrt_p folded into the
   rhs) so the PE consumes etc_k chunks straight off the DMA.
 - biases enter as K=1 matmul accumulation steps; compiled out when zero.
"""
import sys

sys.path.insert(0, "/opt/trn_rl_repo")

import ml_dtypes
import numpy as np

import concourse.mybir as mybir
import concourse.tile as tile
from concourse import bacc, bass_utils

HEAD, DIM, ETC = 8, 256, 512
BAT, SEQ = 4, 1024
NCORES = 8
HPC = HEAD // 2          # heads per core = 4
HD = HPC * DIM           # head-dim columns per core = 1024
P = 128
SC = SEQ // P            # 8 s-tiles
NS = SEQ // 512          # 2 s free-dim chunks
DT = DIM // P            # 2 d-tiles
ET = ETC // P            # 4 e-tiles
KC = HD // P             # 8 hd chunks
F32 = mybir.dt.float32
F32R = mybir.dt.float32r
BF16 = mybir.dt.bfloat16

_NC = {}


def _build(use_collective=True, with_bias=False):
    nc = bacc.Bacc("TRN2", target_bir_lowering=False, debug=False,
                   num_devices=NCORES if use_collective else 1)

    def din(name, shape, dt=F32R):
        return nc.dram_tensor(name, shape, dt, kind="ExternalInput").ap()

    y_d = din("y", [DIM, SEQ])                    # y[b]  [c, s]
    est_d = din("est", [HPC, P, SC, ETC], BF16)   # e_s[h,b].T as [p, sc, e]
    maskreg_d = din("maskreg", [1, SEQ], F32R)    # mask[b]*regular
    vmask_d = din("vmask", [P, SC], F32)          # mask (hg0) / ones, per s-tile
    mixsp_d = din("mixsp", [P, DT * HPC], F32)    # mix[h,d]/sqrt_p, col h*DT+dt
    qwt_d = din("qwt", [DIM, DIM])                # q_w.T
    vwt_d = din("vwt", [DIM, HD])                 # v_w[head rows].T
    rewt_d = din("rewt", [HD, DIM])               # re_w[:, head cols].T
    sqwt_d = din("sqwt", [HD, DIM])               # sq_w[:, head cols].T
    w2t_d = din("w2t", [DIM, DIM])                # avgpool-branch weight, .T
    ones_d = din("ones", [1, 512])                # literal ones
    ident_d = din("ident", [P, P])                # identity for PE transpose
    if with_bias:
        qb_d = din("qb", [1, DIM])
        vb_d = din("vb", [1, HD])
        rebh_d = din("rebh", [1, DIM])            # re_b / 2
        sqbh_d = din("sqbh", [1, DIM])            # sq_b / 2
    out_d = nc.dram_tensor("out", [DIM, SEQ], F32, kind="ExternalOutput").ap()

    with tile.TileContext(nc) as tc:
        with (
            tc.tile_pool(name="const", bufs=1) as cpool,
            tc.tile_pool(name="big", bufs=1) as big,
            tc.tile_pool(name="est", bufs=4) as estp,
            tc.tile_pool(name="work1", bufs=1) as work1,
            tc.tile_pool(name="work2", bufs=2) as work2,
            tc.tile_pool(name="ps", bufs=5, space="PSUM") as psp,
            tc.tile_pool(name="psb", bufs=3, space="PSUM") as psb,
            tc.tile_pool(name="dram", bufs=4, space="DRAM") as dram,
        ):
            # ---- DMA priority order (single in-order SP queue) ----
            def cload(tag, dram_ap, shape, dt=F32R, rearr=None):
                t = cpool.tile(shape, dt, tag=tag)
                nc.sync.dma_start(t[:], dram_ap if rearr is None
                                  else dram_ap.rearrange(rearr, p=P))
                return t

            y_s = cpool.tile([P, DT, SEQ], F32R, tag="y")
            vwt_s = cpool.tile([P, DT, HD], F32R, tag="vwt")
            vmask_s = cload("vmask", vmask_d, [P, SC], F32)
            maskrow = cload("maskrow", maskreg_d, [1, SEQ])
            nc.sync.dma_start(vwt_s[:, 0, 0:512], vwt_d[0:P, 0:512])
            nc.sync.dma_start(y_s[:, 0, 0:512], y_d[0:P, 0:512])
            nc.sync.dma_start(y_s[:, 1, 0:512], y_d[P:DIM, 0:512])
            nc.sync.dma_start(vwt_s[:, 1, 0:512], vwt_d[P:DIM, 0:512])
            nc.sync.dma_start(y_s[:, 0, 512:SEQ], y_d[0:P, 512:SEQ])
            nc.sync.dma_start(y_s[:, 1, 512:SEQ], y_d[P:DIM, 512:SEQ])
            nc.sync.dma_start(vwt_s[:, 0, 512:1024], vwt_d[0:P, 512:1024])
            nc.sync.dma_start(vwt_s[:, 1, 512:1024], vwt_d[P:DIM, 512:1024])

            # est/attenU live as half tiles: 4 slots of 8KB/partition, so a
            # head's first half frees (and the next prefetch starts) midway
            # through its compute instead of at the end
            est_half = {}

            def load_est(h):
                a = estp.tile([P, SC // 2, ETC], BF16, tag="est",
                              name=f"est{h}a")
                b = estp.tile([P, SC // 2, ETC], BF16, tag="est",
                              name=f"est{h}b")
                est_half[h] = (a, b)
                nc.sync.dma_start(a[:], est_d[h, :, 0:SC // 2])
                nc.sync.dma_start(b[:], est_d[h, :, SC // 2:SC])

            def est_sc(h, sc):
                return est_half[h][sc // (SC // 2)][:, sc % (SC // 2), :]

            load_est(0)
            load_est(1)
            rewt_s = cload("rewt", rewt_d, [P, KC, DIM],
                           rearr="(t p) o -> p t o")
            load_est(2)
            load_est(3)
            qwt_s = cload("qwt", qwt_d, [P, DT, DIM], rearr="(t p) o -> p t o")
            w2t_s = cload("w2t", w2t_d, [P, DT, DIM], rearr="(t p) o -> p t o")
            mixsp_s = cload("mixsp", mixsp_d, [P, DT * HPC], F32)
            ones_row = cload("ones_row", ones_d, [1, 512])
            ones_full = cpool.tile([P, P], F32R, tag="ones_full")
            nc.sync.dma_start(ones_full[:],
                              ones_d[:, 0:P].to_broadcast((P, P)))
            ident_s = cload("ident", ident_d, [P, P])
            if with_bias:
                qb_s = cload("qb", qb_d, [1, DIM])
                vb_s = cload("vb", vb_d, [1, HD])
                rebh_s = cload("rebh", rebh_d, [1, DIM])
                sqbh_s = cload("sqbh", sqbh_d, [1, DIM])
            sqwt_s = cload("sqwt", sqwt_d, [P, KC, DIM],
                           rearr="(t p) o -> p t o")
            # (AllReduce chain DMAs + out stores are emitted inline below;
            # mt=0 leg rides the now-empty SP queue, mt=1 the Pool swdge.)

            # warm the PE immediately (zero x zero into the first real psum
            # group; exact) so the p-state ramp completes before real work
            wz = cpool.tile([P, 512], F32R, tag="wz")
            actw = cpool.tile([1, 16], F32, tag="actw")
            nc.vector.memset(wz[:], 0.0)
            nc.vector.memset(actw[:], 0.0)
            # touch ACT immediately so its LoadActFuncSet (~1.3us) runs off
            # the critical path; writes scratch so the warmup's wz read has
            # no dependency on it
            nc.scalar.activation(actw[:, 0:1], wz[0:1, 0:1],
                                 mybir.ActivationFunctionType.Copy)

            # ---- vT[s, hd] = Y.T @ v_wT (+ v_b); head-0 columns masked ----
            # nj-split: the first matmul only needs vwt00+y0a off the wire
            vt = big.tile([P, SC, HD], BF16, tag="vt")
            for nj in range(HD // 512):
                for st in range(SC):
                    ps = psp.tile([P, 512], F32, tag="mm",
                                  name=f"psv{nj}_{st}")
                    first = (nj == 0 and st == 0)
                    if first:
                        for w in range(8):
                            nc.tensor.matmul(ps[:], lhsT=wz[:, 0:P],
                                             rhs=wz[:], start=(w == 0),
                                             stop=False)
                    last = DT - 1 if not with_bias else None
                    for kt in range(DT):
                        nc.tensor.matmul(
                            ps[:], lhsT=y_s[:, kt, st * P:(st + 1) * P],
                            rhs=vwt_s[:, kt, nj * 512:(nj + 1) * 512],
                            start=(kt == 0 and not first),
                            stop=(kt == last))
                    if with_bias:
                        nc.tensor.matmul(
                            ps[:], lhsT=ones_row[:, 0:P],
                            rhs=vb_s[:, nj * 512:(nj + 1) * 512],
                            start=False, stop=True)
                    if nj == 0:
                        nc.vector.tensor_scalar_mul(
                            vt[:, st, 0:DIM], ps[:, 0:DIM],
                            vmask_s[:, st:st + 1])
                        nc.scalar.activation(
                            vt[:, st, DIM:512], ps[:, DIM:512],
                            mybir.ActivationFunctionType.Copy)
                    else:
                        nc.scalar.activation(
                            vt[:, st, 512:1024], ps[:],
                            mybir.ActivationFunctionType.Copy)

            # ---- per head: etc_v[d,e]; rk accumulation interleaved so the
            #      AllReduce can launch right behind the last etc_v ----
            etcv = cpool.tile([P, DT * HPC, ETC], F32R, tag="etcv")
            etcvt = cpool.tile([P, ET * HPC, DIM], F32R, tag="etcvt")
            rk_ps = [psp.tile([P, 512], F32, tag="mm", name=f"rkps{mt}")
                     for mt in range(DT)]

            def mk_etcv(h):
                psv = [psp.tile([P, 512], F32, tag="mm", name=f"psv{h}_{j}")
                       for j in range(DT)]
                for sc in range(SC):
                    for mt in range(DT):
                        nc.tensor.matmul(
                            psv[mt][:],
                            lhsT=vt[:, sc,
                                    h * DIM + mt * P:h * DIM + (mt + 1) * P],
                            rhs=est_sc(h, sc),
                            start=(sc == 0), stop=(sc == SC - 1))
                for mt in range(DT):
                    nc.scalar.activation(etcv[:, h * DT + mt, :], psv[mt][:],
                                         mybir.ActivationFunctionType.Copy)

            def rk_steps(h):
                for mt in range(DT):
                    for kc in (2 * h, 2 * h + 1):
                        nc.tensor.matmul(
                            rk_ps[mt][:],
                            lhsT=rewt_s[:, kc, mt * P:(mt + 1) * P],
                            rhs=etcv[:, kc, :], start=(kc == 0),
                            stop=(kc == KC - 1 and not with_bias))
                if with_bias and h == HPC - 1:
                    for mt in range(DT):
                        nc.tensor.matmul(
                            rk_ps[mt][:], lhsT=rebh_s[:, mt * P:(mt + 1) * P],
                            rhs=ones_row[:], start=False, stop=True)

            mk_etcv(0)
            mk_etcv(1)
            rk_steps(0)
            mk_etcv(2)
            rk_steps(1)
            mk_etcv(3)
            rk_steps(2)
            rk_steps(3)

            # ---- AllReduce, chunked per d-tile across two DMA queues ----
            # bf16 chain: halves every hop of store -> AllReduce -> load;
            # etc_k's ~0.2% rounding stays well inside the error budget and
            # bf16 lhsT runs the PE at the same 1 cycle/row
            rkbuf = work1.tile([P, DT, ETC], BF16, tag="rk")
            arin = [dram.tile([P, ETC], BF16, tag="arin", name=f"arin{mt}")
                    for mt in range(DT)]
            arout = [dram.tile([P, ETC], BF16, tag="arout", name=f"arout{mt}")
                     for mt in range(DT)]
            etck = work1.tile([P, DT, ETC], BF16, tag="etck")
            # parallel psum->sbuf drains: mt0 on DVE, mt1 on ACT
            nc.vector.tensor_copy(out=rkbuf[:, 0, :], in_=rk_ps[0][:])
            nc.scalar.activation(rkbuf[:, 1, :], rk_ps[1][:],
                                 mybir.ActivationFunctionType.Copy)
            nc.sync.dma_start(arin[0][:], rkbuf[:, 0, :])
            nc.gpsimd.dma_start(arin[1][:], rkbuf[:, 1, :])
            if use_collective:
                for mt in range(DT):
                    nc.gpsimd.collective_compute(
                        "AllReduce", mybir.AluOpType.add,
                        replica_groups=[[0, 1], [2, 3], [4, 5], [6, 7]],
                        ins=[arin[mt].opt()], outs=[arout[mt].opt()])
            else:  # timing-model stand-in for TimelineSim (no collectives)
                for mt in range(DT):
                    nc.gpsimd.dma_start(arout[mt][:], arin[mt][:])
            nc.sync.dma_start(etck[:, 0, :], arout[0][:])
            nc.gpsimd.dma_start(etck[:, 1, :], arout[1][:])

            # ---- AllReduce shadow: etc_vT via PE transposes (pairs packed
            #      per PSUM bank), mask broadcast, QM, avgpool branch ----
            for h in range(HPC):
                for et2 in range(ET // 2):
                    pst = psp.tile([P, 512], F32R, tag="mm",
                                   name=f"ptr{h}_{et2}")
                    for ei in range(2):
                        for dt_ in range(DT):
                            et = et2 * 2 + ei
                            nc.tensor.transpose(
                                pst[:, ei * DIM + dt_ * P:
                                    ei * DIM + (dt_ + 1) * P],
                                etcv[:, h * DT + dt_, et * P:(et + 1) * P],
                                ident_s[:])
                    nc.scalar.activation(
                        etcvt[:, h * ET + et2 * 2:h * ET + et2 * 2 + 2, :],
                        pst[:], mybir.ActivationFunctionType.Copy)

            # maskbc[p, s] = maskreg broadcast across partitions (K=1 matmul)
            maskbc = cpool.tile([P, SEQ], F32, tag="maskbc")
            for sj in range(NS):
                psm = psp.tile([P, 512], F32, tag="mm", name=f"psm{sj}")
                nc.tensor.matmul(psm[:], lhsT=ones_row[:, 0:P],
                                 rhs=maskrow[:, sj * 512:(sj + 1) * 512],
                                 start=True, stop=True)
                nc.vector.tensor_copy(out=maskbc[:, sj * 512:(sj + 1) * 512],
                                      in_=psm[:])

            # QM[d, s] = (q_wT.T @ Y (+ q_b)) * maskreg; head-0 rhs variant
            # qmh0 = QM * mix[h0]/sqrt_p so scores(h0) reads etck directly
            qm = cpool.tile([P, DT, SEQ], F32R, tag="qm")
            qmh0 = cpool.tile([P, DT, SEQ], F32R, tag="qmh0")
            for mt in range(DT):
                pss = [psp.tile([P, 512], F32, tag="mm", name=f"psq{mt}_{j}")
                       for j in range(NS)]
                last = DT - 1 if not with_bias else None
                for kt in range(DT):
                    for sj in range(NS):
                        nc.tensor.matmul(
                            pss[sj][:], lhsT=qwt_s[:, kt, mt * P:(mt + 1) * P],
                            rhs=y_s[:, kt, sj * 512:(sj + 1) * 512],
                            start=(kt == 0), stop=(kt == last))
                for sj in range(NS):
                    if with_bias:
                        nc.tensor.matmul(
                            pss[sj][:], lhsT=qb_s[:, mt * P:(mt + 1) * P],
                            rhs=ones_row[:], start=False, stop=True)
                    nc.vector.tensor_tensor(
                        out=qm[:, mt, sj * 512:(sj + 1) * 512],
                        in0=pss[sj][:],
                        in1=maskbc[:, sj * 512:(sj + 1) * 512],
                        op=mybir.AluOpType.mult)
                nc.vector.tensor_scalar_mul(
                    qmh0[:, mt, :], qm[:, mt, :], mixsp_s[:, mt:mt + 1])

            # avgpool branch: P2 = W2T.T @ QM, then 3-tap shift-add (Pool)
            p2s = cpool.tile([P, DT, SEQ + 2], F32, tag="p2s")
            nc.vector.memset(p2s[:, :, 0:1], 0.0)
            nc.vector.memset(p2s[:, :, SEQ + 1:SEQ + 2], 0.0)
            for mt in range(DT):
                pss = [psp.tile([P, 512], F32, tag="mm", name=f"psp{mt}_{j}")
                       for j in range(NS)]
                for kt in range(DT):
                    for sj in range(NS):
                        nc.tensor.matmul(
                            pss[sj][:], lhsT=w2t_s[:, kt, mt * P:(mt + 1) * P],
                            rhs=qm[:, kt, sj * 512:(sj + 1) * 512],
                            start=(kt == 0), stop=(kt == DT - 1))
                for sj in range(NS):
                    nc.scalar.activation(
                        p2s[:, mt, 1 + sj * 512:1 + (sj + 1) * 512],
                        pss[sj][:], mybir.ActivationFunctionType.Copy)
            sum3 = cpool.tile([P, DT, SEQ], F32, tag="sum3")

            # ---- attention, software-pipelined one unit ahead ----
            attnout = big.tile([P, SC, HD], F32R, tag="vt")  # reuses vt slot
            attenU_t = {}

            # attention, software-pipelined one head ahead: big 16-matmul
            # groups keep the PE saturated (the p-state model punishes any
            # drain with a slow-clock restart)
            etckh_t = {}

            def scores_head(h):
                if h == 0:
                    lhs, rhs = etck, qmh0
                else:
                    etckh = work2.tile([P, DT, ETC], F32R, tag="etckh",
                                       name=f"etckh{h}")
                    for dt_ in range(DT):
                        nc.vector.tensor_scalar_mul(
                            etckh[:, dt_, :], etck[:, dt_, :],
                            mixsp_s[:, h * DT + dt_:h * DT + dt_ + 1])
                    lhs, rhs = etckh, qm
                aU = [estp.tile([P, ET, 512], F32R, tag="est",
                                name=f"attenU{h}_{j}") for j in range(NS)]
                attenU_t[h] = aU
                for et in range(ET):
                    pss = [psp.tile([P, 512], F32, tag="mm",
                                    name=f"pss{h}_{et}_{j}")
                           for j in range(NS)]
                    for kt in range(DT):
                        for sj in range(NS):
                            nc.tensor.matmul(
                                pss[sj][:],
                                lhsT=lhs[:, kt, et * P:(et + 1) * P],
                                rhs=rhs[:, kt, sj * 512:(sj + 1) * 512],
                                start=(kt == 0), stop=(kt == DT - 1))
                    for sj in range(NS):
                        nc.scalar.activation(
                            aU[sj][:, et, :],
                            pss[sj][:], mybir.ActivationFunctionType.Exp)

            def z_attnout_head(h):
                aU = attenU_t[h]
                # pre-reduce the four e-tiles pairwise (one add on Pool, one
                # on DVE, in parallel), then a 2-step accumulated full-ones
                # matmul replicates the cross-partition sum.  bf16 pair-sums:
                # Z adds them exactly in PSUM across 128 partitions, so the
                # 0.2% element rounding averages down ~11x.
                zs = work2.tile([P, NS, 512], F32, tag="zs", name=f"zs{h}")
                for sj in range(NS):
                    zt = work2.tile([P, 2, 512], BF16, tag="zt",
                                    name=f"zt{h}_{sj}")
                    nc.gpsimd.tensor_tensor(out=zt[:, 0, :],
                                            in0=aU[sj][:, 0, :],
                                            in1=aU[sj][:, 1, :],
                                            op=mybir.AluOpType.add)
                    nc.vector.tensor_tensor(out=zt[:, 1, :],
                                            in0=aU[sj][:, 2, :],
                                            in1=aU[sj][:, 3, :],
                                            op=mybir.AluOpType.add)
                    psz = psb.tile([P, 512], F32, tag="zb",
                                   name=f"psz{h}_{sj}")
                    nc.tensor.matmul(psz[:], lhsT=ones_full[:],
                                     rhs=zt[:, 0, :], start=True, stop=False)
                    nc.tensor.matmul(psz[:], lhsT=ones_full[:],
                                     rhs=zt[:, 1, :], start=False, stop=True)
                    nc.vector.reciprocal(out=zs[:, sj, :], in_=psz[:])
                for mt in range(DT):
                    pss = [psb.tile([P, 512], F32, tag="zb",
                                    name=f"psa{h}_{mt}_{j}")
                           for j in range(NS)]
                    for et in range(ET):
                        for sj in range(NS):
                            nc.tensor.matmul(
                                pss[sj][:],
                                lhsT=etcvt[:, h * ET + et,
                                           mt * P:(mt + 1) * P],
                                rhs=aU[sj][:, et, :],
                                start=(et == 0), stop=(et == ET - 1))
                    for sj in range(NS):
                        nc.vector.tensor_tensor(
                            out=attnout[:, h * DT + mt,
                                        sj * 512:(sj + 1) * 512],
                            in0=pss[sj][:],
                            in1=zs[:, sj, :],
                            op=mybir.AluOpType.mult)

            # ---- final partial: sq_wT.T @ attnout (+ sq_b/2) + sum3 ----
            fin3 = cpool.tile([P, DT, SEQ + 2], F32, tag="p2s")  # p2s slot
            fin = fin3[:, :, 0:SEQ]

            def fin_sj(sj):
                for mt in range(DT):
                    ps = psp.tile([P, 512], F32, tag="mm",
                                  name=f"psf{mt}_{sj}")
                    last = KC - 1 if not with_bias else None
                    for kc in range(KC):
                        nc.tensor.matmul(
                            ps[:], lhsT=sqwt_s[:, kc, mt * P:(mt + 1) * P],
                            rhs=attnout[:, kc, sj * 512:(sj + 1) * 512],
                            start=(kc == 0), stop=(kc == last))
                    if with_bias:
                        nc.tensor.matmul(
                            ps[:], lhsT=sqbh_s[:, mt * P:(mt + 1) * P],
                            rhs=ones_row[:], start=False, stop=True)
                    nc.vector.tensor_tensor(
                        out=fin[:, mt, sj * 512:(sj + 1) * 512],
                        in0=ps[:],
                        in1=sum3[:, mt, sj * 512:(sj + 1) * 512],
                        op=mybir.AluOpType.add)
                    nc.sync.dma_start(
                        out_d[mt * P:(mt + 1) * P, sj * 512:(sj + 1) * 512],
                        fin[:, mt, sj * 512:(sj + 1) * 512])

            sum3_parts = []

            def mk_sum3_parts():
                for mt in range(DT):
                    sum3_parts.append(lambda mt=mt: nc.gpsimd.tensor_tensor(
                        out=sum3[:, mt, :], in0=p2s[:, mt, 0:SEQ],
                        in1=p2s[:, mt, 1:SEQ + 1], op=mybir.AluOpType.add))
                    sum3_parts.append(lambda mt=mt: nc.gpsimd.tensor_tensor(
                        out=sum3[:, mt, :], in0=sum3[:, mt, :],
                        in1=p2s[:, mt, 2:SEQ + 2], op=mybir.AluOpType.add))

            mk_sum3_parts()
            scores_head(0)
            for h in range(HPC):
                if h + 1 < HPC:
                    scores_head(h + 1)
                z_attnout_head(h)
                # spread the long sum3 Pool ops behind the per-head pair-sums
                if h < 2:
                    sum3_parts[2 * h]()
                    sum3_parts[2 * h + 1]()
            fin_sj(0)
            fin_sj(1)

    nc.compile()
    return nc


def _prep_inputs(y, e_s, mask, regular, mix, sqrt_p, q_w, q_b, v_w, v_b,
                 re_w, re_b, sq_w, sq_b, with_bias=False):
    f = np.float32
    y = np.asarray(y, f)
    e_s = np.asarray(e_s, f)
    mask = np.asarray(mask, f)
    reg = float(np.asarray(regular))
    mix = np.asarray(mix, f)
    sp = float(np.asarray(sqrt_p))
    q_w, q_b = np.asarray(q_w, f), np.asarray(q_b, f)
    v_w, v_b = np.asarray(v_w, f), np.asarray(v_b, f)
    re_w, re_b = np.asarray(re_w, f), np.asarray(re_b, f)
    sq_w, sq_b = np.asarray(sq_w, f), np.asarray(sq_b, f)

    qwt = np.ascontiguousarray(q_w.T)
    in_maps = []
    for c in range(NCORES):
        b, hg = c // 2, c % 2
        hh = slice(hg * HPC, hg * HPC + HPC)
        hd = slice(hg * HD, hg * HD + HD)
        # [h, s, e] -> [h, p, sc, e] with s = sc*P + p (contiguous per
        # partition for max DMA efficiency)
        est = np.ascontiguousarray(
            e_s[hh, b].transpose(0, 2, 1).reshape(HPC, SC, P, ETC)
            .transpose(0, 2, 1, 3)).astype(ml_dtypes.bfloat16)
        maskreg = (mask[b, 0] * reg).astype(f)[None]
        vm = maskreg[0] if hg == 0 else np.ones(SEQ, f)
        vmask = np.ascontiguousarray(vm.reshape(SC, P).T)
        mxs = (mix[hh, :, 0] / sp).astype(f)            # [HPC, DIM]
        mixsp = np.ascontiguousarray(
            mxs.reshape(HPC, DT, P).transpose(2, 0, 1).reshape(P, HPC * DT))
        # W2[o,d] = (1/3) sum_{h in hh} sq_w[o, h*DIM+d] * mix[h,d]
        sqw_h = sq_w.reshape(DIM, HEAD, DIM)[:, hh]      # [o, HPC, d]
        w2 = (sqw_h * mix[hh, :, 0][None]).sum(1) / 3.0  # [o, d]
        m = {
            "y": np.ascontiguousarray(y[b]),
            "est": est,
            "maskreg": maskreg,
            "vmask": vmask,
            "mixsp": mixsp,
            "qwt": qwt,
            "vwt": np.ascontiguousarray(v_w[hd].T),
            "rewt": np.ascontiguousarray(re_w[:, hd].T),
            "sqwt": np.ascontiguousarray(sq_w[:, hd].T),
            "w2t": np.ascontiguousarray(w2.T.astype(f)),
            "ones": np.ones((1, 512), f),
            "ident": np.eye(P, dtype=f),
        }
        if with_bias:
            m.update({
                "qb": np.ascontiguousarray(q_b[None]),
                "vb": np.ascontiguousarray(v_b[hd][None]),
                "rebh": np.ascontiguousarray((re_b / 2)[None]),
                "sqbh": np.ascontiguousarray((sq_b / 2)[None]),
            })
        in_maps.append(m)
    return in_maps


def kernel(**inputs):
    with_bias = any(
        float(np.abs(np.asarray(inputs[k])).max()) != 0.0
        for k in ("q_b", "v_b", "re_b", "sq_b"))
    key = ("hw", with_bias)
    if key not in _NC:
        _NC[key] = _build(use_collective=True, with_bias=with_bias)
    in_maps = _prep_inputs(**inputs, with_bias=with_bias)
    try:
        res = bass_utils.run_bass_kernel_spmd(_NC[key], in_maps,
                                              core_ids=list(range(NCORES)))
    except Exception:
        # the axon tunnel occasionally drops a worker; settle and retry once
        import time
        time.sleep(5)
        res = bass_utils.run_bass_kernel_spmd(_NC[key], in_maps,
                                              core_ids=list(range(NCORES)))
    out = np.empty((BAT, DIM, SEQ), np.float32)
    for b in range(BAT):
        out[b] = res.results[2 * b]["out"] + res.results[2 * b + 1]["out"]
    return out


# revision 16
# speedup vs baseline: 1.0950x; 1.0950x over previous
"""Trainium2 Bass kernel for nn_Block_39247411151159.

Sharding: 8 cores = 4 batches x 2 head-groups (4 heads each).
Core c handles batch b=c//2, head-group hg=c%2 (global heads 4*hg..4*hg+3).
One pairwise AllReduce mid-kernel sums the re-atten conv partials (etc_k);
the final squeeze-conv partials are summed on the host during unshard.

All matmuls run as float32r (TF32-like: full PE speed at N>=256, ~1e-4
relative error). Softmax over the etc axis runs with e on partitions and no
max-subtraction (scores here are O(2)); the denominator is pre-reduced over
the four e-tiles on the (otherwise idle) Pool engine, then replicated across
partitions by a single full-ones stationary matmul per 512-column chunk.

Schedule notes (all per-core):
 - vT runs nj-split so the first matmul only needs 2 DMA chunks; tiny
   zero-warmup matmuls start the PE p-state ramp at ~250ns.
 - etc_v runs per head with the re-conv (rk) accumulation steps interleaved
   right behind each head, so the AllReduce launches ~3us after the last
   etc_v matmul.  The AllReduce is chunked per d-tile over two DMA queues
   (SP hwdge + Pool swdge) to halve exposed issue latency.
 - etc_vT for all four heads is produced by PE transposes (pairs packed per
   PSUM bank) inside the AllReduce shadow, alongside the q conv, the
   avgpool-branch conv, and a K=1 ones-matmul that replicates mask*regular
   across partitions (replaces a pathologically slow broadcast DMA).
 - head 0's scores use a pre-scaled copy of QM (mix/sqrt_p folded into the
   rhs) so the PE consumes etc_k chunks straight off the DMA.
 - biases enter as K=1 matmul accumulation steps; compiled out when zero.
"""
import sys

sys.path.insert(0, "/opt/trn_rl_repo")

import ml_dtypes
import numpy as np

import concourse.mybir as mybir
import concourse.tile as tile
from concourse import bacc, bass_utils

HEAD, DIM, ETC = 8, 256, 512
BAT, SEQ = 4, 1024
NCORES = 8
HPC = HEAD // 2          # heads per core = 4
HD = HPC * DIM           # head-dim columns per core = 1024
P = 128
SC = SEQ // P            # 8 s-tiles
NS = SEQ // 512          # 2 s free-dim chunks
DT = DIM // P            # 2 d-tiles
ET = ETC // P            # 4 e-tiles
KC = HD // P             # 8 hd chunks
F32 = mybir.dt.float32
F32R = mybir.dt.float32r
BF16 = mybir.dt.bfloat16

_NC = {}


def _build(use_collective=True, with_bias=False):
    nc = bacc.Bacc("TRN2", target_bir_lowering=False, debug=False,
                   num_devices=NCORES if use_collective else 1)

    def din(name, shape, dt=F32R):
        return nc.dram_tensor(name, shape, dt, kind="ExternalInput").ap()

    y_d = din("y", [DIM, SEQ])                    # y[b]  [c, s]
    est_d = din("est", [HPC, P, SC, ETC], BF16)   # e_s[h,b].T as [p, sc, e]
    maskreg_d = din("maskreg", [1, SEQ], F32R)    # mask[b]*regular
    vmask_d = din("vmask", [P, SC], F32)          # mask (hg0) / ones, per s-tile
    mixsp_d = din("mixsp", [P, DT * HPC], F32)    # mix[h,d]/sqrt_p, col h*DT+dt
    qwt_d = din("qwt", [DIM, DIM])                # q_w.T
    vwt_d = din("vwt", [DIM, HD])                 # v_w[head rows].T
    rewt_d = din("rewt", [HD, DIM])               # re_w[:, head cols].T
    sqwt_d = din("sqwt", [HD, DIM])               # sq_w[:, head cols].T
    w2t_d = din("w2t", [DIM, DIM])                # avgpool-branch weight, .T
    ones_d = din("ones", [1, 512])                # literal ones
    ident_d = din("ident", [P, P])                # identity for PE transpose
    if with_bias:
        qb_d = din("qb", [1, DIM])
        vb_d = din("vb", [1, HD])
        rebh_d = din("rebh", [1, DIM])            # re_b / 2
        sqbh_d = din("sqbh", [1, DIM])            # sq_b / 2
    out_d = nc.dram_tensor("out", [DIM, SEQ], F32, kind="ExternalOutput").ap()

    with tile.TileContext(nc) as tc:
        with (
            tc.tile_pool(name="const", bufs=1) as cpool,
            tc.tile_pool(name="big", bufs=1) as big,
            tc.tile_pool(name="est", bufs=4) as estp,
            tc.tile_pool(name="work1", bufs=1) as work1,
            tc.tile_pool(name="work2", bufs=2) as work2,
            tc.tile_pool(name="ps", bufs=8, space="PSUM") as psp,
            tc.tile_pool(name="dram", bufs=4, space="DRAM") as dram,
        ):
            # ---- DMA priority order (single in-order SP queue) ----
            def cload(tag, dram_ap, shape, dt=F32R, rearr=None):
                t = cpool.tile(shape, dt, tag=tag)
                nc.sync.dma_start(t[:], dram_ap if rearr is None
                                  else dram_ap.rearrange(rearr, p=P))
                return t

            y_s = cpool.tile([P, DT, SEQ], F32R, tag="y")
            vwt_s = cpool.tile([P, DT, HD], F32R, tag="vwt")
            vmask_s = cload("vmask", vmask_d, [P, SC], F32)
            maskrow = cload("maskrow", maskreg_d, [1, SEQ])
            nc.sync.dma_start(vwt_s[:, 0, 0:512], vwt_d[0:P, 0:512])
            nc.sync.dma_start(y_s[:, 0, 0:512], y_d[0:P, 0:512])
            nc.sync.dma_start(y_s[:, 1, 0:512], y_d[P:DIM, 0:512])
            nc.sync.dma_start(vwt_s[:, 1, 0:512], vwt_d[P:DIM, 0:512])
            nc.sync.dma_start(y_s[:, 0, 512:SEQ], y_d[0:P, 512:SEQ])
            nc.sync.dma_start(y_s[:, 1, 512:SEQ], y_d[P:DIM, 512:SEQ])
            nc.sync.dma_start(vwt_s[:, 0, 512:1024], vwt_d[0:P, 512:1024])
            nc.sync.dma_start(vwt_s[:, 1, 512:1024], vwt_d[P:DIM, 512:1024])

            # est/attenU live as half tiles: 4 slots of 8KB/partition, so a
            # head's first half frees (and the next prefetch starts) midway
            # through its compute instead of at the end
            est_half = {}

            def load_est(h):
                a = estp.tile([P, SC // 2, ETC], BF16, tag="est",
                              name=f"est{h}a")
                b = estp.tile([P, SC // 2, ETC], BF16, tag="est",
                              name=f"est{h}b")
                est_half[h] = (a, b)
                nc.sync.dma_start(a[:], est_d[h, :, 0:SC // 2])
                nc.sync.dma_start(b[:], est_d[h, :, SC // 2:SC])

            def est_sc(h, sc):
                return est_half[h][sc // (SC // 2)][:, sc % (SC // 2), :]

            load_est(0)
            load_est(1)
            rewt_s = cload("rewt", rewt_d, [P, KC, DIM],
                           rearr="(t p) o -> p t o")
            load_est(2)
            load_est(3)
            qwt_s = cload("qwt", qwt_d, [P, DT, DIM], rearr="(t p) o -> p t o")
            w2t_s = cload("w2t", w2t_d, [P, DT, DIM], rearr="(t p) o -> p t o")
            mixsp_s = cload("mixsp", mixsp_d, [P, DT * HPC], F32)
            ones_row = cload("ones_row", ones_d, [1, 512])
            ones_full = cpool.tile([P, P], F32R, tag="ones_full")
            nc.sync.dma_start(ones_full[:],
                              ones_d[:, 0:P].to_broadcast((P, P)))
            ident_s = cload("ident", ident_d, [P, P])
            if with_bias:
                qb_s = cload("qb", qb_d, [1, DIM])
                vb_s = cload("vb", vb_d, [1, HD])
                rebh_s = cload("rebh", rebh_d, [1, DIM])
                sqbh_s = cload("sqbh", sqbh_d, [1, DIM])
            sqwt_s = cload("sqwt", sqwt_d, [P, KC, DIM],
                           rearr="(t p) o -> p t o")
            # (AllReduce chain DMAs + out stores are emitted inline below;
            # mt=0 leg rides the now-empty SP queue, mt=1 the Pool swdge.)

            # warm the PE immediately (zero x zero into the first real psum
            # group; exact) so the p-state ramp completes before real work
            wz = cpool.tile([P, 512], F32R, tag="wz")
            actw = cpool.tile([1, 16], F32, tag="actw")
            nc.vector.memset(wz[:], 0.0)
            nc.vector.memset(actw[:], 0.0)
            # touch ACT immediately so its LoadActFuncSet (~1.3us) runs off
            # the critical path; writes scratch so the warmup's wz read has
            # no dependency on it
            nc.scalar.activation(actw[:, 0:1], wz[0:1, 0:1],
                                 mybir.ActivationFunctionType.Copy)

            # ---- vT[s, hd] = Y.T @ v_wT (+ v_b); head-0 columns masked ----
            # nj-split: the first matmul only needs vwt00+y0a off the wire
            vt = big.tile([P, SC, HD], BF16, tag="vt")
            for nj in range(HD // 512):
                for st in range(SC):
                    ps = psp.tile([P, 512], F32, tag="mm",
                                  name=f"psv{nj}_{st}")
                    first = (nj == 0 and st == 0)
                    if first:
                        for w in range(8):
                            nc.tensor.matmul(ps[:], lhsT=wz[:, 0:P],
                                             rhs=wz[:], start=(w == 0),
                                             stop=False)
                    last = DT - 1 if not with_bias else None
                    for kt in range(DT):
                        nc.tensor.matmul(
                            ps[:], lhsT=y_s[:, kt, st * P:(st + 1) * P],
                            rhs=vwt_s[:, kt, nj * 512:(nj + 1) * 512],
                            start=(kt == 0 and not first),
                            stop=(kt == last))
                    if with_bias:
                        nc.tensor.matmul(
                            ps[:], lhsT=ones_row[:, 0:P],
                            rhs=vb_s[:, nj * 512:(nj + 1) * 512],
                            start=False, stop=True)
                    if nj == 0:
                        nc.vector.tensor_scalar_mul(
                            vt[:, st, 0:DIM], ps[:, 0:DIM],
                            vmask_s[:, st:st + 1])
                        nc.scalar.activation(
                            vt[:, st, DIM:512], ps[:, DIM:512],
                            mybir.ActivationFunctionType.Copy)
                    else:
                        nc.scalar.activation(
                            vt[:, st, 512:1024], ps[:],
                            mybir.ActivationFunctionType.Copy)

            # ---- per head: etc_v[d,e]; rk accumulation interleaved so the
            #      AllReduce can launch right behind the last etc_v ----
            etcv = cpool.tile([P, DT * HPC, ETC], F32R, tag="etcv")
            etcvt = cpool.tile([P, ET * HPC, DIM], F32R, tag="etcvt")
            rk_ps = [psp.tile([P, 512], F32, tag="mm", name=f"rkps{mt}")
                     for mt in range(DT)]

            def mk_etcv(h):
                psv = [psp.tile([P, 512], F32, tag="mm", name=f"psv{h}_{j}")
                       for j in range(DT)]
                for sc in range(SC):
                    for mt in range(DT):
                        nc.tensor.matmul(
                            psv[mt][:],
                            lhsT=vt[:, sc,
                                    h * DIM + mt * P:h * DIM + (mt + 1) * P],
                            rhs=est_sc(h, sc),
                            start=(sc == 0), stop=(sc == SC - 1))
                for mt in range(DT):
                    nc.scalar.activation(etcv[:, h * DT + mt, :], psv[mt][:],
                                         mybir.ActivationFunctionType.Copy)

            def rk_steps(h):
                for mt in range(DT):
                    for kc in (2 * h, 2 * h + 1):
                        nc.tensor.matmul(
                            rk_ps[mt][:],
                            lhsT=rewt_s[:, kc, mt * P:(mt + 1) * P],
                            rhs=etcv[:, kc, :], start=(kc == 0),
                            stop=(kc == KC - 1 and not with_bias))
                if with_bias and h == HPC - 1:
                    for mt in range(DT):
                        nc.tensor.matmul(
                            rk_ps[mt][:], lhsT=rebh_s[:, mt * P:(mt + 1) * P],
                            rhs=ones_row[:], start=False, stop=True)

            mk_etcv(0)
            mk_etcv(1)
            rk_steps(0)
            mk_etcv(2)
            rk_steps(1)
            mk_etcv(3)
            rk_steps(2)
            rk_steps(3)

            # ---- AllReduce, chunked per d-tile across two DMA queues ----
            # bf16 chain: halves every hop of store -> AllReduce -> load;
            # etc_k's ~0.2% rounding stays well inside the error budget and
            # bf16 lhsT runs the PE at the same 1 cycle/row
            rkbuf = work1.tile([P, DT, ETC], BF16, tag="rk")
            arin = [dram.tile([P, ETC], BF16, tag="arin", name=f"arin{mt}")
                    for mt in range(DT)]
            arout = [dram.tile([P, ETC], BF16, tag="arout", name=f"arout{mt}")
                     for mt in range(DT)]
            etck = work1.tile([P, DT, ETC], BF16, tag="etck")
            # parallel psum->sbuf drains: mt0 on DVE, mt1 on ACT
            nc.vector.tensor_copy(out=rkbuf[:, 0, :], in_=rk_ps[0][:])
            nc.scalar.activation(rkbuf[:, 1, :], rk_ps[1][:],
                                 mybir.ActivationFunctionType.Copy)
            nc.sync.dma_start(arin[0][:], rkbuf[:, 0, :])
            nc.gpsimd.dma_start(arin[1][:], rkbuf[:, 1, :])
            if use_collective:
                for mt in range(DT):
                    nc.gpsimd.collective_compute(
                        "AllReduce", mybir.AluOpType.add,
                        replica_groups=[[0, 1], [2, 3], [4, 5], [6, 7]],
                        ins=[arin[mt].opt()], outs=[arout[mt].opt()])
            else:  # timing-model stand-in for TimelineSim (no collectives)
                for mt in range(DT):
                    nc.gpsimd.dma_start(arout[mt][:], arin[mt][:])
            nc.sync.dma_start(etck[:, 0, :], arout[0][:])
            nc.gpsimd.dma_start(etck[:, 1, :], arout[1][:])

            # ---- AllReduce shadow: etc_vT via PE transposes (pairs packed
            #      per PSUM bank), mask broadcast, QM, avgpool branch ----
            for h in range(HPC):
                for et2 in range(ET // 2):
                    pst = psp.tile([P, 512], F32R, tag="mm",
                                   name=f"ptr{h}_{et2}")
                    for ei in range(2):
                        for dt_ in range(DT):
                            et = et2 * 2 + ei
                            nc.tensor.transpose(
                                pst[:, ei * DIM + dt_ * P:
                                    ei * DIM + (dt_ + 1) * P],
                                etcv[:, h * DT + dt_, et * P:(et + 1) * P],
                                ident_s[:])
                    nc.scalar.activation(
                        etcvt[:, h * ET + et2 * 2:h * ET + et2 * 2 + 2, :],
                        pst[:], mybir.ActivationFunctionType.Copy)

            # maskbc[p, s] = maskreg broadcast across partitions (K=1 matmul)
            maskbc = cpool.tile([P, SEQ], F32, tag="maskbc")
            for sj in range(NS):
                psm = psp.tile([P, 512], F32, tag="mm", name=f"psm{sj}")
                nc.tensor.matmul(psm[:], lhsT=ones_row[:, 0:P],
                                 rhs=maskrow[:, sj * 512:(sj + 1) * 512],
                                 start=True, stop=True)
                nc.vector.tensor_copy(out=maskbc[:, sj * 512:(sj + 1) * 512],
                                      in_=psm[:])

            # QM[d, s] = (q_wT.T @ Y (+ q_b)) * maskreg; head-0 rhs variant
            # qmh0 = QM * mix[h0]/sqrt_p so scores(h0) reads etck directly
            qm = cpool.tile([P, DT, SEQ], F32R, tag="qm")
            qmh0 = cpool.tile([P, DT, SEQ], F32R, tag="qmh0")
            for mt in range(DT):
                pss = [psp.tile([P, 512], F32, tag="mm", name=f"psq{mt}_{j}")
                       for j in range(NS)]
                last = DT - 1 if not with_bias else None
                for kt in range(DT):
                    for sj in range(NS):
                        nc.tensor.matmul(
                            pss[sj][:], lhsT=qwt_s[:, kt, mt * P:(mt + 1) * P],
                            rhs=y_s[:, kt, sj * 512:(sj + 1) * 512],
                            start=(kt == 0), stop=(kt == last))
                for sj in range(NS):
                    if with_bias:
                        nc.tensor.matmul(
                            pss[sj][:], lhsT=qb_s[:, mt * P:(mt + 1) * P],
                            rhs=ones_row[:], start=False, stop=True)
                    nc.vector.tensor_tensor(
                        out=qm[:, mt, sj * 512:(sj + 1) * 512],
                        in0=pss[sj][:],
                        in1=maskbc[:, sj * 512:(sj + 1) * 512],
                        op=mybir.AluOpType.mult)
                nc.vector.tensor_scalar_mul(
                    qmh0[:, mt, :], qm[:, mt, :], mixsp_s[:, mt:mt + 1])

            # avgpool branch: P2 = W2T.T @ QM, then 3-tap shift-add (Pool)
            p2s = cpool.tile([P, DT, SEQ + 2], F32, tag="p2s")
            nc.vector.memset(p2s[:, :, 0:1], 0.0)
            nc.vector.memset(p2s[:, :, SEQ + 1:SEQ + 2], 0.0)
            for mt in range(DT):
                pss = [psp.tile([P, 512], F32, tag="mm", name=f"psp{mt}_{j}")
                       for j in range(NS)]
                for kt in range(DT):
                    for sj in range(NS):
                        nc.tensor.matmul(
                            pss[sj][:], lhsT=w2t_s[:, kt, mt * P:(mt + 1) * P],
                            rhs=qm[:, kt, sj * 512:(sj + 1) * 512],
                            start=(kt == 0), stop=(kt == DT - 1))
                for sj in range(NS):
                    nc.scalar.activation(
                        p2s[:, mt, 1 + sj * 512:1 + (sj + 1) * 512],
                        pss[sj][:], mybir.ActivationFunctionType.Copy)
            sum3 = cpool.tile([P, DT, SEQ], F32, tag="sum3")

            # ---- attention, software-pipelined one unit ahead ----
            attnout = big.tile([P, SC, HD], F32R, tag="vt")  # reuses vt slot
            attenU_t = {}

            # attention, software-pipelined one head ahead: big 16-matmul
            # groups keep the PE saturated (the p-state model punishes any
            # drain with a slow-clock restart)
            etckh_t = {}

            def scores_head(h):
                if h == 0:
                    lhs, rhs = etck, qmh0
                else:
                    etckh = work2.tile([P, DT, ETC], F32R, tag="etckh",
                                       name=f"etckh{h}")
                    for dt_ in range(DT):
                        nc.vector.tensor_scalar_mul(
                            etckh[:, dt_, :], etck[:, dt_, :],
                            mixsp_s[:, h * DT + dt_:h * DT + dt_ + 1])
                    lhs, rhs = etckh, qm
                aU = [estp.tile([P, ET, 512], F32R, tag="est",
                                name=f"attenU{h}_{j}") for j in range(NS)]
                attenU_t[h] = aU
                for et in range(ET):
                    pss = [psp.tile([P, 512], F32, tag="mm",
                                    name=f"pss{h}_{et}_{j}")
                           for j in range(NS)]
                    for kt in range(DT):
                        for sj in range(NS):
                            nc.tensor.matmul(
                                pss[sj][:],
                                lhsT=lhs[:, kt, et * P:(et + 1) * P],
                                rhs=rhs[:, kt, sj * 512:(sj + 1) * 512],
                                start=(kt == 0), stop=(kt == DT - 1))
                    for sj in range(NS):
                        nc.scalar.activation(
                            aU[sj][:, et, :],
                            pss[sj][:], mybir.ActivationFunctionType.Exp)

            def z_attnout_head(h):
                aU = attenU_t[h]
                # pre-reduce the four e-tiles pairwise (one add on Pool, one
                # on DVE, in parallel), then a 2-step accumulated full-ones
                # matmul replicates the cross-partition sum.  bf16 pair-sums:
                # Z adds them exactly in PSUM across 128 partitions, so the
                # 0.2% element rounding averages down ~11x.
                zs = work2.tile([P, NS, 512], F32, tag="zs", name=f"zs{h}")
                for sj in range(NS):
                    zt = work2.tile([P, 2, 512], BF16, tag="zt",
                                    name=f"zt{h}_{sj}")
                    nc.gpsimd.tensor_tensor(out=zt[:, 0, :],
                                            in0=aU[sj][:, 0, :],
                                            in1=aU[sj][:, 1, :],
                                            op=mybir.AluOpType.add)
                    nc.vector.tensor_tensor(out=zt[:, 1, :],
                                            in0=aU[sj][:, 2, :],
                                            in1=aU[sj][:, 3, :],
                                            op=mybir.AluOpType.add)
                    psz = psp.tile([P, 512], F32, tag="mm",
                                   name=f"psz{h}_{sj}")
                    nc.tensor.matmul(psz[:], lhsT=ones_full[:],
                                     rhs=zt[:, 0, :], start=True, stop=False)
                    nc.tensor.matmul(psz[:], lhsT=ones_full[:],
                                     rhs=zt[:, 1, :], start=False, stop=True)
                    nc.vector.reciprocal(out=zs[:, sj, :], in_=psz[:])
                for mt in range(DT):
                    pss = [psp.tile([P, 512], F32, tag="mm",
                                    name=f"psa{h}_{mt}_{j}")
                           for j in range(NS)]
                    for et in range(ET):
                        for sj in range(NS):
                            nc.tensor.matmul(
                                pss[sj][:],
                                lhsT=etcvt[:, h * ET + et,
                                           mt * P:(mt + 1) * P],
                                rhs=aU[sj][:, et, :],
                                start=(et == 0), stop=(et == ET - 1))
                    for sj in range(NS):
                        nc.vector.tensor_tensor(
                            out=attnout[:, h * DT + mt,
                                        sj * 512:(sj + 1) * 512],
                            in0=pss[sj][:],
                            in1=zs[:, sj, :],
                            op=mybir.AluOpType.mult)

            # ---- final partial: sq_wT.T @ attnout (+ sq_b/2) + sum3 ----
            fin3 = cpool.tile([P, DT, SEQ + 2], F32, tag="p2s")  # p2s slot
            fin = fin3[:, :, 0:SEQ]

            def fin_sj(sj):
                for mt in range(DT):
                    ps = psp.tile([P, 512], F32, tag="mm",
                                  name=f"psf{mt}_{sj}")
                    last = KC - 1 if not with_bias else None
                    for kc in range(KC):
                        nc.tensor.matmul(
                            ps[:], lhsT=sqwt_s[:, kc, mt * P:(mt + 1) * P],
                            rhs=attnout[:, kc, sj * 512:(sj + 1) * 512],
                            start=(kc == 0), stop=(kc == last))
                    if with_bias:
                        nc.tensor.matmul(
                            ps[:], lhsT=sqbh_s[:, mt * P:(mt + 1) * P],
                            rhs=ones_row[:], start=False, stop=True)
                    nc.vector.tensor_tensor(
                        out=fin[:, mt, sj * 512:(sj + 1) * 512],
                        in0=ps[:],
                        in1=sum3[:, mt, sj * 512:(sj + 1) * 512],
                        op=mybir.AluOpType.add)
                    nc.sync.dma_start(
                        out_d[mt * P:(mt + 1) * P, sj * 512:(sj + 1) * 512],
                        fin[:, mt, sj * 512:(sj + 1) * 512])

            sum3_parts = []

            def mk_sum3_parts():
                for mt in range(DT):
                    sum3_parts.append(lambda mt=mt: nc.gpsimd.tensor_tensor(
                        out=sum3[:, mt, :], in0=p2s[:, mt, 0:SEQ],
                        in1=p2s[:, mt, 1:SEQ + 1], op=mybir.AluOpType.add))
                    sum3_parts.append(lambda mt=mt: nc.gpsimd.tensor_tensor(
                        out=sum3[:, mt, :], in0=sum3[:, mt, :],
                        in1=p2s[:, mt, 2:SEQ + 2], op=mybir.AluOpType.add))

            mk_sum3_parts()
            scores_head(0)
            for h in range(HPC):
                if h + 1 < HPC:
                    scores_head(h + 1)
                z_attnout_head(h)
                # spread the long sum3 Pool ops behind the per-head pair-sums
                if h < 2:
                    sum3_parts[2 * h]()
                    sum3_parts[2 * h + 1]()
            fin_sj(0)
            fin_sj(1)

    nc.compile()
    return nc


def _prep_inputs(y, e_s, mask, regular, mix, sqrt_p, q_w, q_b, v_w, v_b,
                 re_w, re_b, sq_w, sq_b, with_bias=False):
    f = np.float32
    y = np.asarray(y, f)
    e_s = np.asarray(e_s, f)
    mask = np.asarray(mask, f)
    reg = float(np.asarray(regular))
    mix = np.asarray(mix, f)
    sp = float(np.asarray(sqrt_p))
    q_w, q_b = np.asarray(q_w, f), np.asarray(q_b, f)
    v_w, v_b = np.asarray(v_w, f), np.asarray(v_b, f)
    re_w, re_b = np.asarray(re_w, f), np.asarray(re_b, f)
    sq_w, sq_b = np.asarray(sq_w, f), np.asarray(sq_b, f)

    qwt = np.ascontiguousarray(q_w.T)
    in_maps = []
    for c in range(NCORES):
        b, hg = c // 2, c % 2
        hh = slice(hg * HPC, hg * HPC + HPC)
        hd = slice(hg * HD, hg * HD + HD)
        # [h, s, e] -> [h, p, sc, e] with s = sc*P + p (contiguous per
        # partition for max DMA efficiency)
        est = np.ascontiguousarray(
            e_s[hh, b].transpose(0, 2, 1).reshape(HPC, SC, P, ETC)
            .transpose(0, 2, 1, 3)).astype(ml_dtypes.bfloat16)
        maskreg = (mask[b, 0] * reg).astype(f)[None]
        vm = maskreg[0] if hg == 0 else np.ones(SEQ, f)
        vmask = np.ascontiguousarray(vm.reshape(SC, P).T)
        mxs = (mix[hh, :, 0] / sp).astype(f)            # [HPC, DIM]
        mixsp = np.ascontiguousarray(
            mxs.reshape(HPC, DT, P).transpose(2, 0, 1).reshape(P, HPC * DT))
        # W2[o,d] = (1/3) sum_{h in hh} sq_w[o, h*DIM+d] * mix[h,d]
        sqw_h = sq_w.reshape(DIM, HEAD, DIM)[:, hh]      # [o, HPC, d]
        w2 = (sqw_h * mix[hh, :, 0][None]).sum(1) / 3.0  # [o, d]
        m = {
            "y": np.ascontiguousarray(y[b]),
            "est": est,
            "maskreg": maskreg,
            "vmask": vmask,
            "mixsp": mixsp,
            "qwt": qwt,
            "vwt": np.ascontiguousarray(v_w[hd].T),
            "rewt": np.ascontiguousarray(re_w[:, hd].T),
            "sqwt": np.ascontiguousarray(sq_w[:, hd].T),
            "w2t": np.ascontiguousarray(w2.T.astype(f)),
            "ones": np.ones((1, 512), f),
            "ident": np.eye(P, dtype=f),
        }
        if with_bias:
            m.update({
                "qb": np.ascontiguousarray(q_b[None]),
                "vb": np.ascontiguousarray(v_b[hd][None]),
                "rebh": np.ascontiguousarray((re_b / 2)[None]),
                "sqbh": np.ascontiguousarray((sq_b / 2)[None]),
            })
        in_maps.append(m)
    return in_maps


def kernel(**inputs):
    with_bias = any(
        float(np.abs(np.asarray(inputs[k])).max()) != 0.0
        for k in ("q_b", "v_b", "re_b", "sq_b"))
    key = ("hw", with_bias)
    if key not in _NC:
        _NC[key] = _build(use_collective=True, with_bias=with_bias)
    in_maps = _prep_inputs(**inputs, with_bias=with_bias)
    try:
        res = bass_utils.run_bass_kernel_spmd(_NC[key], in_maps,
                                              core_ids=list(range(NCORES)))
    except Exception:
        # the axon tunnel occasionally drops a worker; settle and retry once
        import time
        time.sleep(5)
        res = bass_utils.run_bass_kernel_spmd(_NC[key], in_maps,
                                              core_ids=list(range(NCORES)))
    out = np.empty((BAT, DIM, SEQ), np.float32)
    for b in range(BAT):
        out[b] = res.results[2 * b]["out"] + res.results[2 * b + 1]["out"]
    return out


# revision 17
# speedup vs baseline: 1.0976x; 1.0025x over previous
"""Trainium2 Bass kernel for nn_Block_39247411151159.

Sharding: 8 cores = 4 batches x 2 head-groups (4 heads each).
Core c handles batch b=c//2, head-group hg=c%2 (global heads 4*hg..4*hg+3).
One pairwise AllReduce mid-kernel sums the re-atten conv partials (etc_k);
the final squeeze-conv partials are summed on the host during unshard.

All matmuls run as float32r (TF32-like: full PE speed at N>=256, ~1e-4
relative error). Softmax over the etc axis runs with e on partitions and no
max-subtraction (scores here are O(2)); the denominator is pre-reduced over
the four e-tiles on the (otherwise idle) Pool engine, then replicated across
partitions by a single full-ones stationary matmul per 512-column chunk.

Schedule notes (all per-core):
 - vT runs nj-split so the first matmul only needs 2 DMA chunks; tiny
   zero-warmup matmuls start the PE p-state ramp at ~250ns.
 - etc_v runs per head with the re-conv (rk) accumulation steps interleaved
   right behind each head, so the AllReduce launches ~3us after the last
   etc_v matmul.  The AllReduce is chunked per d-tile over two DMA queues
   (SP hwdge + Pool swdge) to halve exposed issue latency.
 - etc_vT for all four heads is produced by PE transposes (pairs packed per
   PSUM bank) inside the AllReduce shadow, alongside the q conv, the
   avgpool-branch conv, and a K=1 ones-matmul that replicates mask*regular
   across partitions (replaces a pathologically slow broadcast DMA).
 - head 0's scores use a pre-scaled copy of QM (mix/sqrt_p folded into the
   rhs) so the PE consumes etc_k chunks straight off the DMA.
 - biases enter as K=1 matmul accumulation steps; compiled out when zero.
"""
import sys

sys.path.insert(0, "/opt/trn_rl_repo")

import ml_dtypes
import numpy as np

import concourse.mybir as mybir
import concourse.tile as tile
from concourse import bacc, bass_utils

HEAD, DIM, ETC = 8, 256, 512
BAT, SEQ = 4, 1024
NCORES = 8
HPC = HEAD // 2          # heads per core = 4
HD = HPC * DIM           # head-dim columns per core = 1024
P = 128
SC = SEQ // P            # 8 s-tiles
NS = SEQ // 512          # 2 s free-dim chunks
DT = DIM // P            # 2 d-tiles
ET = ETC // P            # 4 e-tiles
KC = HD // P             # 8 hd chunks
F32 = mybir.dt.float32
F32R = mybir.dt.float32r
BF16 = mybir.dt.bfloat16

_NC = {}


def _build(use_collective=True, with_bias=False):
    nc = bacc.Bacc("TRN2", target_bir_lowering=False, debug=False,
                   num_devices=NCORES if use_collective else 1)

    def din(name, shape, dt=F32R):
        return nc.dram_tensor(name, shape, dt, kind="ExternalInput").ap()

    y_d = din("y", [DIM, SEQ])                    # y[b]  [c, s]
    est_d = din("est", [HPC, P, SC, ETC], BF16)   # e_s[h,b].T as [p, sc, e]
    maskreg_d = din("maskreg", [1, SEQ], F32R)    # mask[b]*regular
    vmask_d = din("vmask", [P, SC], F32)          # mask (hg0) / ones, per s-tile
    mixsp_d = din("mixsp", [P, DT * HPC], F32)    # mix[h,d]/sqrt_p, col h*DT+dt
    qwt_d = din("qwt", [DIM, DIM])                # q_w.T
    vwt_d = din("vwt", [DIM, HD])                 # v_w[head rows].T
    rewt_d = din("rewt", [HD, DIM])               # re_w[:, head cols].T
    sqwt_d = din("sqwt", [HD, DIM])               # sq_w[:, head cols].T
    w2t_d = din("w2t", [DIM, DIM])                # avgpool-branch weight, .T
    ones_d = din("ones", [1, 512])                # literal ones
    ident_d = din("ident", [P, P])                # identity for PE transpose
    if with_bias:
        qb_d = din("qb", [1, DIM])
        vb_d = din("vb", [1, HD])
        rebh_d = din("rebh", [1, DIM])            # re_b / 2
        sqbh_d = din("sqbh", [1, DIM])            # sq_b / 2
    out_d = nc.dram_tensor("out", [DIM, SEQ], F32, kind="ExternalOutput").ap()

    with tile.TileContext(nc) as tc:
        with (
            tc.tile_pool(name="const", bufs=1) as cpool,
            tc.tile_pool(name="big", bufs=1) as big,
            tc.tile_pool(name="est", bufs=4) as estp,
            tc.tile_pool(name="work1", bufs=1) as work1,
            tc.tile_pool(name="work2", bufs=2) as work2,
            tc.tile_pool(name="ps", bufs=8, space="PSUM") as psp,
            tc.tile_pool(name="dram", bufs=4, space="DRAM") as dram,
        ):
            # ---- DMA priority order (single in-order SP queue) ----
            def cload(tag, dram_ap, shape, dt=F32R, rearr=None):
                t = cpool.tile(shape, dt, tag=tag)
                nc.sync.dma_start(t[:], dram_ap if rearr is None
                                  else dram_ap.rearrange(rearr, p=P))
                return t

            y_s = cpool.tile([P, DT, SEQ], F32R, tag="y")
            vwt_s = cpool.tile([P, DT, HD], F32R, tag="vwt")
            vmask_s = cload("vmask", vmask_d, [P, SC], F32)
            maskrow = cload("maskrow", maskreg_d, [1, SEQ])
            nc.sync.dma_start(vwt_s[:, 0, 0:512], vwt_d[0:P, 0:512])
            nc.sync.dma_start(y_s[:, 0, 0:512], y_d[0:P, 0:512])
            nc.sync.dma_start(y_s[:, 1, 0:512], y_d[P:DIM, 0:512])
            nc.sync.dma_start(vwt_s[:, 1, 0:512], vwt_d[P:DIM, 0:512])
            nc.sync.dma_start(y_s[:, 0, 512:SEQ], y_d[0:P, 512:SEQ])
            nc.sync.dma_start(y_s[:, 1, 512:SEQ], y_d[P:DIM, 512:SEQ])
            nc.sync.dma_start(vwt_s[:, 0, 512:1024], vwt_d[0:P, 512:1024])
            nc.sync.dma_start(vwt_s[:, 1, 512:1024], vwt_d[P:DIM, 512:1024])

            # est/attenU live as half tiles: 4 slots of 8KB/partition, so a
            # head's first half frees (and the next prefetch starts) midway
            # through its compute instead of at the end
            est_half = {}

            def load_est(h):
                a = estp.tile([P, SC // 2, ETC], BF16, tag="est",
                              name=f"est{h}a")
                b = estp.tile([P, SC // 2, ETC], BF16, tag="est",
                              name=f"est{h}b")
                est_half[h] = (a, b)
                nc.sync.dma_start(a[:], est_d[h, :, 0:SC // 2])
                nc.sync.dma_start(b[:], est_d[h, :, SC // 2:SC])

            def est_sc(h, sc):
                return est_half[h][sc // (SC // 2)][:, sc % (SC // 2), :]

            load_est(0)
            load_est(1)
            rewt_s = cload("rewt", rewt_d, [P, KC, DIM],
                           rearr="(t p) o -> p t o")
            load_est(2)
            load_est(3)
            qwt_s = cload("qwt", qwt_d, [P, DT, DIM], rearr="(t p) o -> p t o")
            w2t_s = cload("w2t", w2t_d, [P, DT, DIM], rearr="(t p) o -> p t o")
            mixsp_s = cload("mixsp", mixsp_d, [P, DT * HPC], F32)
            ones_row = cload("ones_row", ones_d, [1, 512])
            ones_full = cpool.tile([P, P], F32R, tag="ones_full")
            nc.sync.dma_start(ones_full[:],
                              ones_d[:, 0:P].to_broadcast((P, P)))
            ident_s = cload("ident", ident_d, [P, P])
            if with_bias:
                qb_s = cload("qb", qb_d, [1, DIM])
                vb_s = cload("vb", vb_d, [1, HD])
                rebh_s = cload("rebh", rebh_d, [1, DIM])
                sqbh_s = cload("sqbh", sqbh_d, [1, DIM])
            sqwt_s = cload("sqwt", sqwt_d, [P, KC, DIM],
                           rearr="(t p) o -> p t o")
            # (AllReduce chain DMAs + out stores are emitted inline below;
            # mt=0 leg rides the now-empty SP queue, mt=1 the Pool swdge.)

            # warm the PE immediately (zero x zero into the first real psum
            # group; exact) so the p-state ramp completes before real work
            wz = cpool.tile([P, 512], F32R, tag="wz")
            actw = cpool.tile([1, 16], F32, tag="actw")
            nc.vector.memset(wz[:], 0.0)
            nc.vector.memset(actw[:], 0.0)
            # touch ACT immediately so its LoadActFuncSet (~1.3us) runs off
            # the critical path; writes scratch so the warmup's wz read has
            # no dependency on it
            nc.scalar.activation(actw[:, 0:1], wz[0:1, 0:1],
                                 mybir.ActivationFunctionType.Copy)

            # ---- vT[s, hd] = Y.T @ v_wT (+ v_b); head-0 columns masked ----
            # nj-split: the first matmul only needs vwt00+y0a off the wire
            vt = big.tile([P, SC, HD], BF16, tag="vt")
            for nj in range(HD // 512):
                for st in range(SC):
                    ps = psp.tile([P, 512], F32, tag="mm",
                                  name=f"psv{nj}_{st}")
                    first = (nj == 0 and st == 0)
                    if first:
                        for w in range(8):
                            nc.tensor.matmul(ps[:], lhsT=wz[:, 0:P],
                                             rhs=wz[:], start=(w == 0),
                                             stop=False)
                    last = DT - 1 if not with_bias else None
                    for kt in range(DT):
                        nc.tensor.matmul(
                            ps[:], lhsT=y_s[:, kt, st * P:(st + 1) * P],
                            rhs=vwt_s[:, kt, nj * 512:(nj + 1) * 512],
                            start=(kt == 0 and not first),
                            stop=(kt == last))
                    if with_bias:
                        nc.tensor.matmul(
                            ps[:], lhsT=ones_row[:, 0:P],
                            rhs=vb_s[:, nj * 512:(nj + 1) * 512],
                            start=False, stop=True)
                    if nj == 0:
                        nc.vector.tensor_scalar_mul(
                            vt[:, st, 0:DIM], ps[:, 0:DIM],
                            vmask_s[:, st:st + 1])
                        nc.scalar.activation(
                            vt[:, st, DIM:512], ps[:, DIM:512],
                            mybir.ActivationFunctionType.Copy)
                    else:
                        nc.scalar.activation(
                            vt[:, st, 512:1024], ps[:],
                            mybir.ActivationFunctionType.Copy)

            # ---- per head: etc_v[d,e]; rk accumulation interleaved so the
            #      AllReduce can launch right behind the last etc_v ----
            etcv = cpool.tile([P, DT * HPC, ETC], F32R, tag="etcv")
            etcvt = cpool.tile([P, ET * HPC, DIM], F32R, tag="etcvt")
            rk_ps = [psp.tile([P, 512], F32, tag="mm", name=f"rkps{mt}")
                     for mt in range(DT)]

            def mk_etcv(h):
                psv = [psp.tile([P, 512], F32, tag="mm", name=f"psv{h}_{j}")
                       for j in range(DT)]
                for sc in range(SC):
                    for mt in range(DT):
                        nc.tensor.matmul(
                            psv[mt][:],
                            lhsT=vt[:, sc,
                                    h * DIM + mt * P:h * DIM + (mt + 1) * P],
                            rhs=est_sc(h, sc),
                            start=(sc == 0), stop=(sc == SC - 1))
                for mt in range(DT):
                    nc.scalar.activation(etcv[:, h * DT + mt, :], psv[mt][:],
                                         mybir.ActivationFunctionType.Copy)

            def rk_steps(h):
                for mt in range(DT):
                    for kc in (2 * h, 2 * h + 1):
                        nc.tensor.matmul(
                            rk_ps[mt][:],
                            lhsT=rewt_s[:, kc, mt * P:(mt + 1) * P],
                            rhs=etcv[:, kc, :], start=(kc == 0),
                            stop=(kc == KC - 1 and not with_bias))
                if with_bias and h == HPC - 1:
                    for mt in range(DT):
                        nc.tensor.matmul(
                            rk_ps[mt][:], lhsT=rebh_s[:, mt * P:(mt + 1) * P],
                            rhs=ones_row[:], start=False, stop=True)

            mk_etcv(0)
            mk_etcv(1)
            rk_steps(0)
            mk_etcv(2)
            rk_steps(1)
            mk_etcv(3)
            rk_steps(2)
            rk_steps(3)

            # ---- AllReduce, chunked per d-tile across two DMA queues ----
            # bf16 chain: halves every hop of store -> AllReduce -> load;
            # etc_k's ~0.2% rounding stays well inside the error budget and
            # bf16 lhsT runs the PE at the same 1 cycle/row
            rkbuf = work1.tile([P, DT, ETC], BF16, tag="rk")
            arin = [dram.tile([P, ETC], BF16, tag="arin", name=f"arin{mt}")
                    for mt in range(DT)]
            arout = [dram.tile([P, ETC], BF16, tag="arout", name=f"arout{mt}")
                     for mt in range(DT)]
            etck = work1.tile([P, DT, ETC], BF16, tag="etck")
            # parallel psum->sbuf drains: mt0 on DVE, mt1 on ACT
            nc.vector.tensor_copy(out=rkbuf[:, 0, :], in_=rk_ps[0][:])
            nc.scalar.activation(rkbuf[:, 1, :], rk_ps[1][:],
                                 mybir.ActivationFunctionType.Copy)
            nc.sync.dma_start(arin[0][:], rkbuf[:, 0, :])
            nc.gpsimd.dma_start(arin[1][:], rkbuf[:, 1, :])
            if use_collective:
                for mt in range(DT):
                    nc.gpsimd.collective_compute(
                        "AllReduce", mybir.AluOpType.add,
                        replica_groups=[[0, 1], [2, 3], [4, 5], [6, 7]],
                        ins=[arin[mt].opt()], outs=[arout[mt].opt()])
            else:  # timing-model stand-in for TimelineSim (no collectives)
                for mt in range(DT):
                    nc.gpsimd.dma_start(arout[mt][:], arin[mt][:])
            nc.sync.dma_start(etck[:, 0, :], arout[0][:])
            nc.gpsimd.dma_start(etck[:, 1, :], arout[1][:])

            # ---- AllReduce shadow: etc_vT via PE transposes (pairs packed
            #      per PSUM bank), mask broadcast, QM, avgpool branch ----
            for h in range(HPC):
                for et2 in range(ET // 2):
                    pst = psp.tile([P, 512], F32R, tag="mm",
                                   name=f"ptr{h}_{et2}")
                    for ei in range(2):
                        for dt_ in range(DT):
                            et = et2 * 2 + ei
                            nc.tensor.transpose(
                                pst[:, ei * DIM + dt_ * P:
                                    ei * DIM + (dt_ + 1) * P],
                                etcv[:, h * DT + dt_, et * P:(et + 1) * P],
                                ident_s[:])
                    nc.scalar.activation(
                        etcvt[:, h * ET + et2 * 2:h * ET + et2 * 2 + 2, :],
                        pst[:], mybir.ActivationFunctionType.Copy)

            # maskbc[p, s] = maskreg broadcast across partitions (K=1 matmul)
            maskbc = cpool.tile([P, SEQ], F32, tag="maskbc")
            for sj in range(NS):
                psm = psp.tile([P, 512], F32, tag="mm", name=f"psm{sj}")
                nc.tensor.matmul(psm[:], lhsT=ones_row[:, 0:P],
                                 rhs=maskrow[:, sj * 512:(sj + 1) * 512],
                                 start=True, stop=True)
                nc.vector.tensor_copy(out=maskbc[:, sj * 512:(sj + 1) * 512],
                                      in_=psm[:])

            # QM[d, s] = (q_wT.T @ Y (+ q_b)) * maskreg; head-0 rhs variant
            # qmh0 = QM * mix[h0]/sqrt_p so scores(h0) reads etck directly
            qm = cpool.tile([P, DT, SEQ], F32R, tag="qm")
            qmh0 = cpool.tile([P, DT, SEQ], F32R, tag="qmh0")
            for mt in range(DT):
                pss = [psp.tile([P, 512], F32, tag="mm", name=f"psq{mt}_{j}")
                       for j in range(NS)]
                last = DT - 1 if not with_bias else None
                for kt in range(DT):
                    for sj in range(NS):
                        nc.tensor.matmul(
                            pss[sj][:], lhsT=qwt_s[:, kt, mt * P:(mt + 1) * P],
                            rhs=y_s[:, kt, sj * 512:(sj + 1) * 512],
                            start=(kt == 0), stop=(kt == last))
                for sj in range(NS):
                    if with_bias:
                        nc.tensor.matmul(
                            pss[sj][:], lhsT=qb_s[:, mt * P:(mt + 1) * P],
                            rhs=ones_row[:], start=False, stop=True)
                    nc.vector.tensor_tensor(
                        out=qm[:, mt, sj * 512:(sj + 1) * 512],
                        in0=pss[sj][:],
                        in1=maskbc[:, sj * 512:(sj + 1) * 512],
                        op=mybir.AluOpType.mult)
                nc.vector.tensor_scalar_mul(
                    qmh0[:, mt, :], qm[:, mt, :], mixsp_s[:, mt:mt + 1])

            # avgpool branch: P2 = W2T.T @ QM, then 3-tap shift-add (Pool)
            p2s = cpool.tile([P, DT, SEQ + 2], F32, tag="p2s")
            nc.vector.memset(p2s[:, :, 0:1], 0.0)
            nc.vector.memset(p2s[:, :, SEQ + 1:SEQ + 2], 0.0)
            for mt in range(DT):
                pss = [psp.tile([P, 512], F32, tag="mm", name=f"psp{mt}_{j}")
                       for j in range(NS)]
                for kt in range(DT):
                    for sj in range(NS):
                        nc.tensor.matmul(
                            pss[sj][:], lhsT=w2t_s[:, kt, mt * P:(mt + 1) * P],
                            rhs=qm[:, kt, sj * 512:(sj + 1) * 512],
                            start=(kt == 0), stop=(kt == DT - 1))
                for sj in range(NS):
                    nc.scalar.activation(
                        p2s[:, mt, 1 + sj * 512:1 + (sj + 1) * 512],
                        pss[sj][:], mybir.ActivationFunctionType.Copy)
            sum3 = cpool.tile([P, DT, SEQ], F32, tag="sum3")

            # ---- attention, software-pipelined one unit ahead ----
            attnout = big.tile([P, SC, HD], F32R, tag="vt")  # reuses vt slot
            attenU_t = {}

            # attention, software-pipelined one head ahead: big 16-matmul
            # groups keep the PE saturated (the p-state model punishes any
            # drain with a slow-clock restart)
            etckh_t = {}

            def scores_head(h):
                if h == 0:
                    lhs, rhs = etck, qmh0
                else:
                    etckh = work2.tile([P, DT, ETC], F32R, tag="etckh",
                                       name=f"etckh{h}")
                    for dt_ in range(DT):
                        nc.vector.tensor_scalar_mul(
                            etckh[:, dt_, :], etck[:, dt_, :],
                            mixsp_s[:, h * DT + dt_:h * DT + dt_ + 1])
                    lhs, rhs = etckh, qm
                aU = [estp.tile([P, ET, 512], F32R, tag="est",
                                name=f"attenU{h}_{j}") for j in range(NS)]
                attenU_t[h] = aU
                for et in range(ET):
                    pss = [psp.tile([P, 512], F32, tag="mm",
                                    name=f"pss{h}_{et}_{j}")
                           for j in range(NS)]
                    for kt in range(DT):
                        for sj in range(NS):
                            nc.tensor.matmul(
                                pss[sj][:],
                                lhsT=lhs[:, kt, et * P:(et + 1) * P],
                                rhs=rhs[:, kt, sj * 512:(sj + 1) * 512],
                                start=(kt == 0), stop=(kt == DT - 1))
                    for sj in range(NS):
                        nc.scalar.activation(
                            aU[sj][:, et, :],
                            pss[sj][:], mybir.ActivationFunctionType.Exp)

            def z_attnout_head(h):
                aU = attenU_t[h]
                # pre-reduce the four e-tiles pairwise on DVE, then a 2-step accumulated full-ones
                # matmul replicates the cross-partition sum.  bf16 pair-sums:
                # Z adds them exactly in PSUM across 128 partitions, so the
                # 0.2% element rounding averages down ~11x.
                zs = work2.tile([P, NS, 512], F32, tag="zs", name=f"zs{h}")
                for sj in range(NS):
                    zt = work2.tile([P, 2, 512], BF16, tag="zt",
                                    name=f"zt{h}_{sj}")
                    nc.vector.tensor_tensor(out=zt[:, 0, :],
                                            in0=aU[sj][:, 0, :],
                                            in1=aU[sj][:, 1, :],
                                            op=mybir.AluOpType.add)
                    nc.vector.tensor_tensor(out=zt[:, 1, :],
                                            in0=aU[sj][:, 2, :],
                                            in1=aU[sj][:, 3, :],
                                            op=mybir.AluOpType.add)
                    psz = psp.tile([P, 512], F32, tag="mm",
                                   name=f"psz{h}_{sj}")
                    nc.tensor.matmul(psz[:], lhsT=ones_full[:],
                                     rhs=zt[:, 0, :], start=True, stop=False)
                    nc.tensor.matmul(psz[:], lhsT=ones_full[:],
                                     rhs=zt[:, 1, :], start=False, stop=True)
                    nc.vector.reciprocal(out=zs[:, sj, :], in_=psz[:])
                for mt in range(DT):
                    pss = [psp.tile([P, 512], F32, tag="mm",
                                    name=f"psa{h}_{mt}_{j}")
                           for j in range(NS)]
                    for et in range(ET):
                        for sj in range(NS):
                            nc.tensor.matmul(
                                pss[sj][:],
                                lhsT=etcvt[:, h * ET + et,
                                           mt * P:(mt + 1) * P],
                                rhs=aU[sj][:, et, :],
                                start=(et == 0), stop=(et == ET - 1))
                    for sj in range(NS):
                        nc.vector.tensor_tensor(
                            out=attnout[:, h * DT + mt,
                                        sj * 512:(sj + 1) * 512],
                            in0=pss[sj][:],
                            in1=zs[:, sj, :],
                            op=mybir.AluOpType.mult)

            # ---- final partial: sq_wT.T @ attnout (+ sq_b/2) + sum3 ----
            fin3 = cpool.tile([P, DT, SEQ + 2], F32, tag="p2s")  # p2s slot
            fin = fin3[:, :, 0:SEQ]

            def fin_sj(sj):
                for mt in range(DT):
                    ps = psp.tile([P, 512], F32, tag="mm",
                                  name=f"psf{mt}_{sj}")
                    last = KC - 1 if not with_bias else None
                    for kc in range(KC):
                        nc.tensor.matmul(
                            ps[:], lhsT=sqwt_s[:, kc, mt * P:(mt + 1) * P],
                            rhs=attnout[:, kc, sj * 512:(sj + 1) * 512],
                            start=(kc == 0), stop=(kc == last))
                    if with_bias:
                        nc.tensor.matmul(
                            ps[:], lhsT=sqbh_s[:, mt * P:(mt + 1) * P],
                            rhs=ones_row[:], start=False, stop=True)
                    nc.vector.tensor_tensor(
                        out=fin[:, mt, sj * 512:(sj + 1) * 512],
                        in0=ps[:],
                        in1=sum3[:, mt, sj * 512:(sj + 1) * 512],
                        op=mybir.AluOpType.add)
                    nc.sync.dma_start(
                        out_d[mt * P:(mt + 1) * P, sj * 512:(sj + 1) * 512],
                        fin[:, mt, sj * 512:(sj + 1) * 512])

            sum3_parts = []

            def mk_sum3_parts():
                for mt in range(DT):
                    sum3_parts.append(lambda mt=mt: nc.gpsimd.tensor_tensor(
                        out=sum3[:, mt, :], in0=p2s[:, mt, 0:SEQ],
                        in1=p2s[:, mt, 1:SEQ + 1], op=mybir.AluOpType.add))
                    sum3_parts.append(lambda mt=mt: nc.gpsimd.tensor_tensor(
                        out=sum3[:, mt, :], in0=sum3[:, mt, :],
                        in1=p2s[:, mt, 2:SEQ + 2], op=mybir.AluOpType.add))

            mk_sum3_parts()
            scores_head(0)
            for h in range(HPC):
                if h + 1 < HPC:
                    scores_head(h + 1)
                z_attnout_head(h)
                # spread the long sum3 Pool ops behind the per-head pair-sums
                if h < 2:
                    sum3_parts[2 * h]()
                    sum3_parts[2 * h + 1]()
            fin_sj(0)
            fin_sj(1)

    nc.compile()
    return nc


def _prep_inputs(y, e_s, mask, regular, mix, sqrt_p, q_w, q_b, v_w, v_b,
                 re_w, re_b, sq_w, sq_b, with_bias=False):
    f = np.float32
    y = np.asarray(y, f)
    e_s = np.asarray(e_s, f)
    mask = np.asarray(mask, f)
    reg = float(np.asarray(regular))
    mix = np.asarray(mix, f)
    sp = float(np.asarray(sqrt_p))
    q_w, q_b = np.asarray(q_w, f), np.asarray(q_b, f)
    v_w, v_b = np.asarray(v_w, f), np.asarray(v_b, f)
    re_w, re_b = np.asarray(re_w, f), np.asarray(re_b, f)
    sq_w, sq_b = np.asarray(sq_w, f), np.asarray(sq_b, f)

    qwt = np.ascontiguousarray(q_w.T)
    in_maps = []
    for c in range(NCORES):
        b, hg = c // 2, c % 2
        hh = slice(hg * HPC, hg * HPC + HPC)
        hd = slice(hg * HD, hg * HD + HD)
        # [h, s, e] -> [h, p, sc, e] with s = sc*P + p (contiguous per
        # partition for max DMA efficiency)
        est = np.ascontiguousarray(
            e_s[hh, b].transpose(0, 2, 1).reshape(HPC, SC, P, ETC)
            .transpose(0, 2, 1, 3)).astype(ml_dtypes.bfloat16)
        maskreg = (mask[b, 0] * reg).astype(f)[None]
        vm = maskreg[0] if hg == 0 else np.ones(SEQ, f)
        vmask = np.ascontiguousarray(vm.reshape(SC, P).T)
        mxs = (mix[hh, :, 0] / sp).astype(f)            # [HPC, DIM]
        mixsp = np.ascontiguousarray(
            mxs.reshape(HPC, DT, P).transpose(2, 0, 1).reshape(P, HPC * DT))
        # W2[o,d] = (1/3) sum_{h in hh} sq_w[o, h*DIM+d] * mix[h,d]
        sqw_h = sq_w.reshape(DIM, HEAD, DIM)[:, hh]      # [o, HPC, d]
        w2 = (sqw_h * mix[hh, :, 0][None]).sum(1) / 3.0  # [o, d]
        m = {
            "y": np.ascontiguousarray(y[b]),
            "est": est,
            "maskreg": maskreg,
            "vmask": vmask,
            "mixsp": mixsp,
            "qwt": qwt,
            "vwt": np.ascontiguousarray(v_w[hd].T),
            "rewt": np.ascontiguousarray(re_w[:, hd].T),
            "sqwt": np.ascontiguousarray(sq_w[:, hd].T),
            "w2t": np.ascontiguousarray(w2.T.astype(f)),
            "ones": np.ones((1, 512), f),
            "ident": np.eye(P, dtype=f),
        }
        if with_bias:
            m.update({
                "qb": np.ascontiguousarray(q_b[None]),
                "vb": np.ascontiguousarray(v_b[hd][None]),
                "rebh": np.ascontiguousarray((re_b / 2)[None]),
                "sqbh": np.ascontiguousarray((sq_b / 2)[None]),
            })
        in_maps.append(m)
    return in_maps


def kernel(**inputs):
    with_bias = any(
        float(np.abs(np.asarray(inputs[k])).max()) != 0.0
        for k in ("q_b", "v_b", "re_b", "sq_b"))
    key = ("hw", with_bias)
    if key not in _NC:
        _NC[key] = _build(use_collective=True, with_bias=with_bias)
    in_maps = _prep_inputs(**inputs, with_bias=with_bias)
    try:
        res = bass_utils.run_bass_kernel_spmd(_NC[key], in_maps,
                                              core_ids=list(range(NCORES)))
    except Exception:
        # the axon tunnel occasionally drops a worker; settle and retry once
        import time
        time.sleep(5)
        res = bass_utils.run_bass_kernel_spmd(_NC[key], in_maps,
                                              core_ids=list(range(NCORES)))
    out = np.empty((BAT, DIM, SEQ), np.float32)
    for b in range(BAT):
        out[b] = res.results[2 * b]["out"] + res.results[2 * b + 1]["out"]
    return out


# revision 18
# speedup vs baseline: 1.1216x; 1.0219x over previous
"""Trainium2 Bass kernel for nn_Block_39247411151159.

Sharding: 8 cores = 4 batches x 2 head-groups (4 heads each).
Core c handles batch b=c//2, head-group hg=c%2 (global heads 4*hg..4*hg+3).
One pairwise AllReduce mid-kernel sums the re-atten conv partials (etc_k);
the final squeeze-conv partials are summed on the host during unshard.

All matmuls run as float32r (TF32-like: full PE speed at N>=256, ~1e-4
relative error). Softmax over the etc axis runs with e on partitions and no
max-subtraction (scores here are O(2)); the denominator is pre-reduced over
the four e-tiles on the (otherwise idle) Pool engine, then replicated across
partitions by a single full-ones stationary matmul per 512-column chunk.

Schedule notes (all per-core):
 - vT runs nj-split so the first matmul only needs 2 DMA chunks; tiny
   zero-warmup matmuls start the PE p-state ramp at ~250ns.
 - etc_v runs per head with the re-conv (rk) accumulation steps interleaved
   right behind each head, so the AllReduce launches ~3us after the last
   etc_v matmul.  The AllReduce is chunked per d-tile over two DMA queues
   (SP hwdge + Pool swdge) to halve exposed issue latency.
 - etc_vT for all four heads is produced by PE transposes (pairs packed per
   PSUM bank) inside the AllReduce shadow, alongside the q conv, the
   avgpool-branch conv, and a K=1 ones-matmul that replicates mask*regular
   across partitions (replaces a pathologically slow broadcast DMA).
 - head 0's scores use a pre-scaled copy of QM (mix/sqrt_p folded into the
   rhs) so the PE consumes etc_k chunks straight off the DMA.
 - biases enter as K=1 matmul accumulation steps; compiled out when zero.
"""
import sys

sys.path.insert(0, "/opt/trn_rl_repo")

import ml_dtypes
import numpy as np

import concourse.mybir as mybir
import concourse.tile as tile
from concourse import bacc, bass_utils

HEAD, DIM, ETC = 8, 256, 512
BAT, SEQ = 4, 1024
NCORES = 8
HPC = HEAD // 2          # heads per core = 4
HD = HPC * DIM           # head-dim columns per core = 1024
P = 128
SC = SEQ // P            # 8 s-tiles
NS = SEQ // 512          # 2 s free-dim chunks
DT = DIM // P            # 2 d-tiles
ET = ETC // P            # 4 e-tiles
KC = HD // P             # 8 hd chunks
F32 = mybir.dt.float32
F32R = mybir.dt.float32r
BF16 = mybir.dt.bfloat16

_NC = {}


def _build(use_collective=True, with_bias=False):
    nc = bacc.Bacc("TRN2", target_bir_lowering=False, debug=False,
                   num_devices=NCORES if use_collective else 1)

    def din(name, shape, dt=F32R):
        return nc.dram_tensor(name, shape, dt, kind="ExternalInput").ap()

    y_d = din("y", [DIM, SEQ])                    # y[b]  [c, s]
    est_d = din("est", [HPC, P, SC, ETC], BF16)   # e_s[h,b].T as [p, sc, e]
    maskreg_d = din("maskreg", [1, SEQ], F32R)    # mask[b]*regular
    vmask_d = din("vmask", [P, SC], F32)          # mask (hg0) / ones, per s-tile
    mixsp_d = din("mixsp", [P, DT * HPC], F32)    # mix[h,d]/sqrt_p, col h*DT+dt
    qwt_d = din("qwt", [DIM, DIM])                # q_w.T
    vwt_d = din("vwt", [DIM, HD])                 # v_w[head rows].T
    rewt_d = din("rewt", [HD, DIM])               # re_w[:, head cols].T
    sqwt_d = din("sqwt", [HD, DIM])               # sq_w[:, head cols].T
    w2t_d = din("w2t", [DIM, DIM])                # avgpool-branch weight, .T
    ones_d = din("ones", [1, 512])                # literal ones
    ident_d = din("ident", [P, P])                # identity for PE transpose
    if with_bias:
        qb_d = din("qb", [1, DIM])
        vb_d = din("vb", [1, HD])
        rebh_d = din("rebh", [1, DIM])            # re_b / 2
        sqbh_d = din("sqbh", [1, DIM])            # sq_b / 2
    out_d = nc.dram_tensor("out", [DIM, SEQ], F32, kind="ExternalOutput").ap()

    with tile.TileContext(nc) as tc:
        with (
            tc.tile_pool(name="const", bufs=1) as cpool,
            tc.tile_pool(name="big", bufs=1) as big,
            tc.tile_pool(name="est", bufs=4) as estp,
            tc.tile_pool(name="work1", bufs=1) as work1,
            tc.tile_pool(name="work2", bufs=2) as work2,
            tc.tile_pool(name="ps", bufs=8, space="PSUM") as psp,
            tc.tile_pool(name="dram", bufs=4, space="DRAM") as dram,
        ):
            # ---- DMA priority order (single in-order SP queue) ----
            def cload(tag, dram_ap, shape, dt=F32R, rearr=None):
                t = cpool.tile(shape, dt, tag=tag)
                nc.sync.dma_start(t[:], dram_ap if rearr is None
                                  else dram_ap.rearrange(rearr, p=P))
                return t

            y_s = cpool.tile([P, DT, SEQ], F32R, tag="y")
            vwt_s = cpool.tile([P, DT, HD], F32R, tag="vwt")
            vmask_s = cload("vmask", vmask_d, [P, SC], F32)
            maskrow = cload("maskrow", maskreg_d, [1, SEQ])
            nc.sync.dma_start(vwt_s[:, 0, 0:512], vwt_d[0:P, 0:512])
            nc.sync.dma_start(y_s[:, 0, 0:512], y_d[0:P, 0:512])
            nc.sync.dma_start(y_s[:, 1, 0:512], y_d[P:DIM, 0:512])
            nc.sync.dma_start(vwt_s[:, 1, 0:512], vwt_d[P:DIM, 0:512])
            nc.sync.dma_start(y_s[:, 0, 512:SEQ], y_d[0:P, 512:SEQ])
            nc.sync.dma_start(y_s[:, 1, 512:SEQ], y_d[P:DIM, 512:SEQ])
            nc.sync.dma_start(vwt_s[:, 0, 512:1024], vwt_d[0:P, 512:1024])
            nc.sync.dma_start(vwt_s[:, 1, 512:1024], vwt_d[P:DIM, 512:1024])

            # est/attenU live as half tiles: 4 slots of 8KB/partition, so a
            # head's first half frees (and the next prefetch starts) midway
            # through its compute instead of at the end
            est_half = {}

            def load_est(h):
                a = estp.tile([P, SC // 2, ETC], BF16, tag="est",
                              name=f"est{h}a")
                b = estp.tile([P, SC // 2, ETC], BF16, tag="est",
                              name=f"est{h}b")
                est_half[h] = (a, b)
                nc.sync.dma_start(a[:], est_d[h, :, 0:SC // 2])
                nc.sync.dma_start(b[:], est_d[h, :, SC // 2:SC])

            def est_sc(h, sc):
                return est_half[h][sc // (SC // 2)][:, sc % (SC // 2), :]

            load_est(0)
            load_est(1)
            rewt_s = cload("rewt", rewt_d, [P, KC, DIM],
                           rearr="(t p) o -> p t o")
            load_est(2)
            load_est(3)
            qwt_s = cload("qwt", qwt_d, [P, DT, DIM], rearr="(t p) o -> p t o")
            w2t_s = cload("w2t", w2t_d, [P, DT, DIM], rearr="(t p) o -> p t o")
            mixsp_s = cload("mixsp", mixsp_d, [P, DT * HPC], F32)
            ones_row = cload("ones_row", ones_d, [1, 512])
            ones_full = cpool.tile([P, P], F32R, tag="ones_full")
            nc.sync.dma_start(ones_full[:],
                              ones_d[:, 0:P].to_broadcast((P, P)))
            ident_s = cload("ident", ident_d, [P, P])
            if with_bias:
                qb_s = cload("qb", qb_d, [1, DIM])
                vb_s = cload("vb", vb_d, [1, HD])
                rebh_s = cload("rebh", rebh_d, [1, DIM])
                sqbh_s = cload("sqbh", sqbh_d, [1, DIM])
            sqwt_s = cload("sqwt", sqwt_d, [P, KC, DIM],
                           rearr="(t p) o -> p t o")
            # (AllReduce chain DMAs + out stores are emitted inline below;
            # mt=0 leg rides the now-empty SP queue, mt=1 the Pool swdge.)

            # warm the PE immediately (zero x zero into the first real psum
            # group; exact) so the p-state ramp completes before real work
            wz = cpool.tile([P, 512], F32R, tag="wz")
            actw = cpool.tile([1, 16], F32, tag="actw")
            nc.vector.memset(wz[:], 0.0)
            nc.vector.memset(actw[:], 0.0)
            # touch ACT immediately so its LoadActFuncSet (~1.3us) runs off
            # the critical path; writes scratch so the warmup's wz read has
            # no dependency on it
            nc.scalar.activation(actw[:, 0:1], wz[0:1, 0:1],
                                 mybir.ActivationFunctionType.Copy)

            # ---- vT[s, hd] = Y.T @ v_wT (+ v_b); head-0 columns masked ----
            # nj-split: the first matmul only needs vwt00+y0a off the wire
            vt = big.tile([P, SC, HD], BF16, tag="vt")
            for nj in range(HD // 512):
                for st in range(SC):
                    ps = psp.tile([P, 512], F32, tag="mm",
                                  name=f"psv{nj}_{st}")
                    first = (nj == 0 and st == 0)
                    if first:
                        for w in range(8):
                            nc.tensor.matmul(ps[:], lhsT=wz[:, 0:P],
                                             rhs=wz[:], start=(w == 0),
                                             stop=False)
                    last = DT - 1 if not with_bias else None
                    for kt in range(DT):
                        nc.tensor.matmul(
                            ps[:], lhsT=y_s[:, kt, st * P:(st + 1) * P],
                            rhs=vwt_s[:, kt, nj * 512:(nj + 1) * 512],
                            start=(kt == 0 and not first),
                            stop=(kt == last))
                    if with_bias:
                        nc.tensor.matmul(
                            ps[:], lhsT=ones_row[:, 0:P],
                            rhs=vb_s[:, nj * 512:(nj + 1) * 512],
                            start=False, stop=True)
                    if nj == 0:
                        nc.vector.tensor_scalar_mul(
                            vt[:, st, 0:DIM], ps[:, 0:DIM],
                            vmask_s[:, st:st + 1])
                        nc.scalar.activation(
                            vt[:, st, DIM:512], ps[:, DIM:512],
                            mybir.ActivationFunctionType.Copy)
                    else:
                        nc.scalar.activation(
                            vt[:, st, 512:1024], ps[:],
                            mybir.ActivationFunctionType.Copy)

            # ---- per head: etc_v[d,e]; rk accumulation interleaved so the
            #      AllReduce can launch right behind the last etc_v ----
            etcv = cpool.tile([P, DT * HPC, ETC], F32R, tag="etcv")
            etcvt = cpool.tile([P, ET * HPC, DIM], F32R, tag="etcvt")
            rk_ps = [psp.tile([P, 512], F32, tag="mm", name=f"rkps{mt}")
                     for mt in range(DT)]

            def mk_etcv(h):
                psv = [psp.tile([P, 512], F32, tag="mm", name=f"psv{h}_{j}")
                       for j in range(DT)]
                for sc in range(SC):
                    for mt in range(DT):
                        nc.tensor.matmul(
                            psv[mt][:],
                            lhsT=vt[:, sc,
                                    h * DIM + mt * P:h * DIM + (mt + 1) * P],
                            rhs=est_sc(h, sc),
                            start=(sc == 0), stop=(sc == SC - 1))
                for mt in range(DT):
                    nc.scalar.activation(etcv[:, h * DT + mt, :], psv[mt][:],
                                         mybir.ActivationFunctionType.Copy)

            def rk_steps(h):
                for mt in range(DT):
                    for kc in (2 * h, 2 * h + 1):
                        nc.tensor.matmul(
                            rk_ps[mt][:],
                            lhsT=rewt_s[:, kc, mt * P:(mt + 1) * P],
                            rhs=etcv[:, kc, :], start=(kc == 0),
                            stop=(kc == KC - 1 and not with_bias))
                if with_bias and h == HPC - 1:
                    for mt in range(DT):
                        nc.tensor.matmul(
                            rk_ps[mt][:], lhsT=rebh_s[:, mt * P:(mt + 1) * P],
                            rhs=ones_row[:], start=False, stop=True)

            mk_etcv(0)
            mk_etcv(1)
            rk_steps(0)
            mk_etcv(2)
            rk_steps(1)
            mk_etcv(3)
            rk_steps(2)
            rk_steps(3)

            # ---- AllReduce, chunked per d-tile across two DMA queues ----
            # bf16 chain: halves every hop of store -> AllReduce -> load;
            # etc_k's ~0.2% rounding stays well inside the error budget and
            # bf16 lhsT runs the PE at the same 1 cycle/row
            rkbuf = work1.tile([P, DT, ETC], BF16, tag="rk")
            arin = [dram.tile([P, ETC], BF16, tag="arin", name=f"arin{mt}")
                    for mt in range(DT)]
            arout = [dram.tile([P, ETC], BF16, tag="arout", name=f"arout{mt}")
                     for mt in range(DT)]
            etck = work1.tile([P, DT, ETC], BF16, tag="etck")
            # parallel psum->sbuf drains: mt0 on DVE, mt1 on ACT
            nc.vector.tensor_copy(out=rkbuf[:, 0, :], in_=rk_ps[0][:])
            nc.scalar.activation(rkbuf[:, 1, :], rk_ps[1][:],
                                 mybir.ActivationFunctionType.Copy)
            nc.sync.dma_start(arin[0][:], rkbuf[:, 0, :])
            nc.gpsimd.dma_start(arin[1][:], rkbuf[:, 1, :])
            if use_collective:
                for mt in range(DT):
                    nc.gpsimd.collective_compute(
                        "AllReduce", mybir.AluOpType.add,
                        replica_groups=[[0, 1], [2, 3], [4, 5], [6, 7]],
                        ins=[arin[mt].opt()], outs=[arout[mt].opt()])
            else:  # timing-model stand-in for TimelineSim (no collectives)
                for mt in range(DT):
                    nc.gpsimd.dma_start(arout[mt][:], arin[mt][:])
            nc.sync.dma_start(etck[:, 0, :], arout[0][:])
            nc.gpsimd.dma_start(etck[:, 1, :], arout[1][:])

            # ---- AllReduce shadow: etc_vT via PE transposes (pairs packed
            #      per PSUM bank), mask broadcast, QM, avgpool branch ----
            for h in range(HPC):
                for et2 in range(ET // 2):
                    pst = psp.tile([P, 512], F32R, tag="mm",
                                   name=f"ptr{h}_{et2}")
                    for ei in range(2):
                        for dt_ in range(DT):
                            et = et2 * 2 + ei
                            nc.tensor.transpose(
                                pst[:, ei * DIM + dt_ * P:
                                    ei * DIM + (dt_ + 1) * P],
                                etcv[:, h * DT + dt_, et * P:(et + 1) * P],
                                ident_s[:])
                    nc.scalar.activation(
                        etcvt[:, h * ET + et2 * 2:h * ET + et2 * 2 + 2, :],
                        pst[:], mybir.ActivationFunctionType.Copy)

            # maskbc[p, s] = maskreg broadcast across partitions (K=1 matmul)
            maskbc = cpool.tile([P, SEQ], F32, tag="maskbc")
            for sj in range(NS):
                psm = psp.tile([P, 512], F32, tag="mm", name=f"psm{sj}")
                nc.tensor.matmul(psm[:], lhsT=ones_row[:, 0:P],
                                 rhs=maskrow[:, sj * 512:(sj + 1) * 512],
                                 start=True, stop=True)
                nc.vector.tensor_copy(out=maskbc[:, sj * 512:(sj + 1) * 512],
                                      in_=psm[:])

            # QM[d, s] = (q_wT.T @ Y (+ q_b)) * maskreg; head-0 rhs variant
            # qmh0 = QM * mix[h0]/sqrt_p so scores(h0) reads etck directly
            qm = cpool.tile([P, DT, SEQ], F32R, tag="qm")
            qmh0 = cpool.tile([P, DT, SEQ], F32R, tag="qmh0")
            for mt in range(DT):
                pss = [psp.tile([P, 512], F32, tag="mm", name=f"psq{mt}_{j}")
                       for j in range(NS)]
                last = DT - 1 if not with_bias else None
                for kt in range(DT):
                    for sj in range(NS):
                        nc.tensor.matmul(
                            pss[sj][:], lhsT=qwt_s[:, kt, mt * P:(mt + 1) * P],
                            rhs=y_s[:, kt, sj * 512:(sj + 1) * 512],
                            start=(kt == 0), stop=(kt == last))
                for sj in range(NS):
                    if with_bias:
                        nc.tensor.matmul(
                            pss[sj][:], lhsT=qb_s[:, mt * P:(mt + 1) * P],
                            rhs=ones_row[:], start=False, stop=True)
                    nc.vector.tensor_tensor(
                        out=qm[:, mt, sj * 512:(sj + 1) * 512],
                        in0=pss[sj][:],
                        in1=maskbc[:, sj * 512:(sj + 1) * 512],
                        op=mybir.AluOpType.mult)
                nc.vector.tensor_scalar_mul(
                    qmh0[:, mt, :], qm[:, mt, :], mixsp_s[:, mt:mt + 1])

            # avgpool branch: P2 = W2T.T @ QM, then 3-tap shift-add (Pool)
            p2s = cpool.tile([P, DT, SEQ + 2], F32, tag="p2s")
            nc.vector.memset(p2s[:, :, 0:1], 0.0)
            nc.vector.memset(p2s[:, :, SEQ + 1:SEQ + 2], 0.0)
            for mt in range(DT):
                pss = [psp.tile([P, 512], F32, tag="mm", name=f"psp{mt}_{j}")
                       for j in range(NS)]
                for kt in range(DT):
                    for sj in range(NS):
                        nc.tensor.matmul(
                            pss[sj][:], lhsT=w2t_s[:, kt, mt * P:(mt + 1) * P],
                            rhs=qm[:, kt, sj * 512:(sj + 1) * 512],
                            start=(kt == 0), stop=(kt == DT - 1))
                for sj in range(NS):
                    nc.scalar.activation(
                        p2s[:, mt, 1 + sj * 512:1 + (sj + 1) * 512],
                        pss[sj][:], mybir.ActivationFunctionType.Copy)
            sum3 = cpool.tile([P, DT, SEQ], F32, tag="sum3")

            # ---- attention, software-pipelined one unit ahead ----
            attnout = big.tile([P, SC, HD], F32R, tag="vt")  # reuses vt slot
            attenU_t = {}

            # attention, software-pipelined one head ahead: big 16-matmul
            # groups keep the PE saturated (the p-state model punishes any
            # drain with a slow-clock restart)
            etckh_t = {}

            def scores_head(h):
                if h == 0:
                    lhs, rhs = etck, qmh0
                else:
                    etckh = work2.tile([P, DT, ETC], F32R, tag="etckh",
                                       name=f"etckh{h}")
                    for dt_ in range(DT):
                        nc.vector.tensor_scalar_mul(
                            etckh[:, dt_, :], etck[:, dt_, :],
                            mixsp_s[:, h * DT + dt_:h * DT + dt_ + 1])
                    lhs, rhs = etckh, qm
                aU = [estp.tile([P, ET, 512], F32R, tag="est",
                                name=f"attenU{h}_{j}") for j in range(NS)]
                attenU_t[h] = aU
                for et in range(ET):
                    pss = [psp.tile([P, 512], F32, tag="mm",
                                    name=f"pss{h}_{et}_{j}")
                           for j in range(NS)]
                    for kt in range(DT):
                        for sj in range(NS):
                            nc.tensor.matmul(
                                pss[sj][:],
                                lhsT=lhs[:, kt, et * P:(et + 1) * P],
                                rhs=rhs[:, kt, sj * 512:(sj + 1) * 512],
                                start=(kt == 0), stop=(kt == DT - 1))
                    for sj in range(NS):
                        nc.scalar.activation(
                            aU[sj][:, et, :],
                            pss[sj][:], mybir.ActivationFunctionType.Exp)

            def z_attnout_head(h):
                aU = attenU_t[h]
                # pre-reduce the four e-tiles pairwise on DVE, then a 2-step accumulated full-ones
                # matmul replicates the cross-partition sum.  bf16 pair-sums:
                # Z adds them exactly in PSUM across 128 partitions, so the
                # 0.2% element rounding averages down ~11x.
                zs = work2.tile([P, NS, 512], F32, tag="zs", name=f"zs{h}")
                for sj in range(NS):
                    zt = work2.tile([P, 3, 512], BF16, tag="zt",
                                    name=f"zt{h}_{sj}")
                    nc.vector.tensor_tensor(out=zt[:, 0, :],
                                            in0=aU[sj][:, 0, :],
                                            in1=aU[sj][:, 1, :],
                                            op=mybir.AluOpType.add)
                    nc.vector.tensor_tensor(out=zt[:, 1, :],
                                            in0=aU[sj][:, 2, :],
                                            in1=aU[sj][:, 3, :],
                                            op=mybir.AluOpType.add)
                    nc.vector.tensor_tensor(out=zt[:, 2, :],
                                            in0=zt[:, 0, :],
                                            in1=zt[:, 1, :],
                                            op=mybir.AluOpType.add)
                    psz = psp.tile([P, 512], F32, tag="mm",
                                   name=f"psz{h}_{sj}")
                    nc.tensor.matmul(psz[:], lhsT=ones_full[:],
                                     rhs=zt[:, 2, :], start=True, stop=True)
                    nc.vector.reciprocal(out=zs[:, sj, :], in_=psz[:])
                for mt in range(DT):
                    pss = [psp.tile([P, 512], F32, tag="mm",
                                    name=f"psa{h}_{mt}_{j}")
                           for j in range(NS)]
                    for et in range(ET):
                        for sj in range(NS):
                            nc.tensor.matmul(
                                pss[sj][:],
                                lhsT=etcvt[:, h * ET + et,
                                           mt * P:(mt + 1) * P],
                                rhs=aU[sj][:, et, :],
                                start=(et == 0), stop=(et == ET - 1))
                    for sj in range(NS):
                        nc.vector.tensor_tensor(
                            out=attnout[:, h * DT + mt,
                                        sj * 512:(sj + 1) * 512],
                            in0=pss[sj][:],
                            in1=zs[:, sj, :],
                            op=mybir.AluOpType.mult)

            # ---- final partial: sq_wT.T @ attnout (+ sq_b/2) + sum3 ----
            fin3 = cpool.tile([P, DT, SEQ + 2], F32, tag="p2s")  # p2s slot
            fin = fin3[:, :, 0:SEQ]

            fin_ps = {}

            def fin_open():
                # heads 0-2 contributions: runs in the last head's exp-drain
                # window; holds 4 open psum banks until fin_close
                for mt in range(DT):
                    for sj in range(NS):
                        ps = psp.tile([P, 512], F32, tag="mm",
                                      name=f"psf{mt}_{sj}")
                        fin_ps[(mt, sj)] = ps
                        for kc in range(2 * (HPC - 1)):
                            nc.tensor.matmul(
                                ps[:],
                                lhsT=sqwt_s[:, kc, mt * P:(mt + 1) * P],
                                rhs=attnout[:, kc, sj * 512:(sj + 1) * 512],
                                start=(kc == 0), stop=False)

            def fin_close(sj):
                for mt in range(DT):
                    ps = fin_ps[(mt, sj)]
                    for kc in range(2 * (HPC - 1), KC):
                        nc.tensor.matmul(
                            ps[:], lhsT=sqwt_s[:, kc, mt * P:(mt + 1) * P],
                            rhs=attnout[:, kc, sj * 512:(sj + 1) * 512],
                            start=False,
                            stop=(kc == KC - 1 and not with_bias))
                    if with_bias:
                        nc.tensor.matmul(
                            ps[:], lhsT=sqbh_s[:, mt * P:(mt + 1) * P],
                            rhs=ones_row[:], start=False, stop=True)
                    nc.vector.tensor_tensor(
                        out=fin[:, mt, sj * 512:(sj + 1) * 512],
                        in0=ps[:],
                        in1=sum3[:, mt, sj * 512:(sj + 1) * 512],
                        op=mybir.AluOpType.add)
                    nc.sync.dma_start(
                        out_d[mt * P:(mt + 1) * P, sj * 512:(sj + 1) * 512],
                        fin[:, mt, sj * 512:(sj + 1) * 512])

            sum3_parts = []

            def mk_sum3_parts():
                for mt in range(DT):
                    sum3_parts.append(lambda mt=mt: nc.gpsimd.tensor_tensor(
                        out=sum3[:, mt, :], in0=p2s[:, mt, 0:SEQ],
                        in1=p2s[:, mt, 1:SEQ + 1], op=mybir.AluOpType.add))
                    sum3_parts.append(lambda mt=mt: nc.gpsimd.tensor_tensor(
                        out=sum3[:, mt, :], in0=sum3[:, mt, :],
                        in1=p2s[:, mt, 2:SEQ + 2], op=mybir.AluOpType.add))

            mk_sum3_parts()
            scores_head(0)
            for h in range(HPC):
                if h + 1 < HPC:
                    scores_head(h + 1)
                z_attnout_head(h)
                # spread the long sum3 Pool ops behind the per-head pair-sums
                if h < 2:
                    sum3_parts[2 * h]()
                    sum3_parts[2 * h + 1]()
                if h == HPC - 2:
                    fin_open()
            fin_close(0)
            fin_close(1)

    nc.compile()
    return nc


def _prep_inputs(y, e_s, mask, regular, mix, sqrt_p, q_w, q_b, v_w, v_b,
                 re_w, re_b, sq_w, sq_b, with_bias=False):
    f = np.float32
    y = np.asarray(y, f)
    e_s = np.asarray(e_s, f)
    mask = np.asarray(mask, f)
    reg = float(np.asarray(regular))
    mix = np.asarray(mix, f)
    sp = float(np.asarray(sqrt_p))
    q_w, q_b = np.asarray(q_w, f), np.asarray(q_b, f)
    v_w, v_b = np.asarray(v_w, f), np.asarray(v_b, f)
    re_w, re_b = np.asarray(re_w, f), np.asarray(re_b, f)
    sq_w, sq_b = np.asarray(sq_w, f), np.asarray(sq_b, f)

    qwt = np.ascontiguousarray(q_w.T)
    in_maps = []
    for c in range(NCORES):
        b, hg = c // 2, c % 2
        hh = slice(hg * HPC, hg * HPC + HPC)
        hd = slice(hg * HD, hg * HD + HD)
        # [h, s, e] -> [h, p, sc, e] with s = sc*P + p (contiguous per
        # partition for max DMA efficiency)
        est = np.ascontiguousarray(
            e_s[hh, b].transpose(0, 2, 1).reshape(HPC, SC, P, ETC)
            .transpose(0, 2, 1, 3)).astype(ml_dtypes.bfloat16)
        maskreg = (mask[b, 0] * reg).astype(f)[None]
        vm = maskreg[0] if hg == 0 else np.ones(SEQ, f)
        vmask = np.ascontiguousarray(vm.reshape(SC, P).T)
        mxs = (mix[hh, :, 0] / sp).astype(f)            # [HPC, DIM]
        mixsp = np.ascontiguousarray(
            mxs.reshape(HPC, DT, P).transpose(2, 0, 1).reshape(P, HPC * DT))
        # W2[o,d] = (1/3) sum_{h in hh} sq_w[o, h*DIM+d] * mix[h,d]
        sqw_h = sq_w.reshape(DIM, HEAD, DIM)[:, hh]      # [o, HPC, d]
        w2 = (sqw_h * mix[hh, :, 0][None]).sum(1) / 3.0  # [o, d]
        m = {
            "y": np.ascontiguousarray(y[b]),
            "est": est,
            "maskreg": maskreg,
            "vmask": vmask,
            "mixsp": mixsp,
            "qwt": qwt,
            "vwt": np.ascontiguousarray(v_w[hd].T),
            "rewt": np.ascontiguousarray(re_w[:, hd].T),
            "sqwt": np.ascontiguousarray(sq_w[:, hd].T),
            "w2t": np.ascontiguousarray(w2.T.astype(f)),
            "ones": np.ones((1, 512), f),
            "ident": np.eye(P, dtype=f),
        }
        if with_bias:
            m.update({
                "qb": np.ascontiguousarray(q_b[None]),
                "vb": np.ascontiguousarray(v_b[hd][None]),
                "rebh": np.ascontiguousarray((re_b / 2)[None]),
                "sqbh": np.ascontiguousarray((sq_b / 2)[None]),
            })
        in_maps.append(m)
    return in_maps


def kernel(**inputs):
    with_bias = any(
        float(np.abs(np.asarray(inputs[k])).max()) != 0.0
        for k in ("q_b", "v_b", "re_b", "sq_b"))
    key = ("hw", with_bias)
    if key not in _NC:
        _NC[key] = _build(use_collective=True, with_bias=with_bias)
    in_maps = _prep_inputs(**inputs, with_bias=with_bias)
    try:
        res = bass_utils.run_bass_kernel_spmd(_NC[key], in_maps,
                                              core_ids=list(range(NCORES)))
    except Exception:
        # the axon tunnel occasionally drops a worker; settle and retry once
        import time
        time.sleep(5)
        res = bass_utils.run_bass_kernel_spmd(_NC[key], in_maps,
                                              core_ids=list(range(NCORES)))
    out = np.empty((BAT, DIM, SEQ), np.float32)
    for b in range(BAT):
        out[b] = res.results[2 * b]["out"] + res.results[2 * b + 1]["out"]
    return out
